# revision 1
# baseline (speedup 1.0000x reference)
"""GAT (2-layer, PyG-style) Trainium2 Bass kernel, 8-core SPMD.

Strategy (see sharding hint): destination-node partitioning. Each core owns a
contiguous range of destination nodes and all edges pointing into it (host
pre-sorts edges by dst block). Per layer:
  - every core computes its node-slice of h = x @ W (plus per-head attention
    logit contributions alpha_src/alpha_dst via host-prefolded W@a columns),
  - AllGather makes the full [N, 320] table (h | a_src | a_dst | pad)
    available to every core,
  - each core streams its edges: dma_gather fetches h[src] rows (1280 B/row),
    attention weights exp(leakyrelu(a_s+a_d)) are computed per edge and folded
    into the gathered rows in place, and a one-hot scatter matrix D (host
    precomputed) turns the segment softmax-weighted aggregation into PSUM
    matmul accumulation; softmax denominators ride along as 4 extra rhs
    columns, so normalization is a cheap post-pass per 128-node block.
Self-loops are added on host. Edge order within a destination block is free,
which lets edges also be grouped by src-half so gather indices fit in int16.
"""

from contextlib import ExitStack

import numpy as np

import concourse.bass as bass
import concourse.bacc as bacc
import concourse.mybir as mybir
import concourse.tile as tile
from concourse.masks import make_identity

P = 128
NC = 8
IN_CH = 16
HEADS = 4
HID = 64
C = HEADS * HID          # 256
OUT_CH = 8
ELEM = 320               # table row: h(256) | a_src(4) | a_dst(4) | pad -> 320 f32
AVW = 64                 # av table row: a_src(4) | a_dst(4) | pad -> 64 f32
NEG_SLOPE = 0.2
F32 = mybir.dt.float32
I16 = mybir.dt.int16


# ----------------------------------------------------------------------------
# host-side preprocessing
# ----------------------------------------------------------------------------

def _wrap16(vals):
    """Pack per-gather-call indices into the [16, n/16] wrapped layout."""
    n = len(vals)
    assert n % 16 == 0
    a = np.zeros((16, n // 16), np.int16)
    a[np.arange(n) % 16, np.arange(n) // 16] = vals.astype(np.int16)
    return a


def _prep_edges(src, dst, n_nodes, npc):
    """Partition edges by dst across cores; group by (dst block, src half).

    Returns meta (shared compile-time structure) and per-core arrays.
    """
    npad = NC * npc
    half = npad // 2
    nb = npc // P                      # node blocks per core
    assert npc % P == 0 and half <= 32768

    core_of = dst // npc
    per_core = []
    counts = np.zeros((NC, nb, 2), np.int64)
    for k in range(NC):
        sel = core_of == k
        s = src[sel]
        dl = dst[sel] - k * npc
        blk = dl >> 7
        hlf = s // half
        order = np.lexsort((hlf, blk))
        s, dl, blk, hlf = s[order], dl[order], blk[order], hlf[order]
        np.add.at(counts[k], (blk, hlf), 1)
        per_core.append((s, dl, blk, hlf))

    # shared tile structure: per (block, half) tile count = max over cores
    T = np.ceil(counts.max(axis=0) / P).astype(np.int64)   # [nb, 2]
    tiles_per_block = T.sum(axis=1)
    tile_start = np.concatenate([[0], np.cumsum(tiles_per_block)])
    TT = int(tile_start[-1])

    meta = {
        "npc": npc, "npad": npad, "half": half, "nb": nb,
        "T": T, "tile_start": tile_start, "TT": TT,
        "tb_max": int(tiles_per_block.max()),
    }

    per_core_arrays = []
    for k in range(NC):
        s, dl, blk, hlf = per_core[k]
        srch = (s % half).astype(np.int64)
        # slot streams
        src_slots = np.zeros(TT * P, np.int64)
        dst_slots = np.zeros(TT * P, np.int64)
        dloc_slots = np.full(TT * P, -1, np.int64)   # -1 = pad slot (zero D row)
        # group boundaries in the sorted edge list
        gstart = np.zeros((nb, 2), np.int64)
        gcount = np.zeros((nb, 2), np.int64)
        idx = 0
        for b in range(nb):
            for h in range(2):
                cnt = int(((blk == b) & (hlf == h)).sum())
                gstart[b, h] = idx
                gcount[b, h] = cnt
                idx += cnt
        pos = 0
        for b in range(nb):
            for h in range(2):
                cnt = int(gcount[b, h])
                g0 = int(gstart[b, h])
                nt = int(T[b, h])
                src_slots[pos:pos + cnt] = srch[g0:g0 + cnt]
                dst_slots[pos:pos + cnt] = dl[g0:g0 + cnt]
                dloc_slots[pos:pos + cnt] = dl[g0:g0 + cnt] & 127
                pos += nt * P
        assert pos == TT * P

        # D one-hot [TT*P, P] f32
        D = np.zeros((TT * P, P), np.float32)
        real = dloc_slots >= 0
        D[np.where(real)[0], dloc_slots[real]] = 1.0

        # per-call wrapped index arrays (col layout: 8 cols per tile slot)
        src_idx = np.zeros((16, 8 * TT), np.int16)
        dst_idx = np.zeros((16, 8 * TT), np.int16)
        for b in range(nb):
            ts0 = int(tile_start[b])
            t0, t1 = int(T[b, 0]), int(T[b, 1])
            if t0:
                sl = slice(ts0 * P, (ts0 + t0) * P)
                src_idx[:, 8 * ts0: 8 * (ts0 + t0)] = _wrap16(src_slots[sl])
            if t1:
                sl = slice((ts0 + t0) * P, (ts0 + t0 + t1) * P)
                src_idx[:, 8 * (ts0 + t0): 8 * (ts0 + t0 + t1)] = _wrap16(src_slots[sl])
            tb = t0 + t1
            if tb:
                sl = slice(ts0 * P, (ts0 + tb) * P)
                dst_idx[:, 8 * ts0: 8 * (ts0 + tb)] = _wrap16(dst_slots[sl])

        per_core_arrays.append({
            "srcidx": np.tile(src_idx, (8, 1)),
            "dstidx": np.tile(dst_idx, (8, 1)),
            "Dmat": D,
        })
    return meta, per_core_arrays


def _fold_weights(W, a_s, a_d):
    """[K, C] -> [K, C+8] with columns C..C+4 = W@As, C+4..C+8 = W@Ad."""
    K = W.shape[0]
    As = np.zeros((C, HEADS), np.float32)
    Ad = np.zeros((C, HEADS), np.float32)
    for h in range(HEADS):
        As[h * HID:(h + 1) * HID, h] = a_s[h]
        Ad[h * HID:(h + 1) * HID, h] = a_d[h]
    return np.concatenate([W, W @ As, W @ Ad], axis=1).astype(np.float32)


# ----------------------------------------------------------------------------
# device program
# ----------------------------------------------------------------------------

def build_gat(tc, outs, ins, meta):
    phases = meta.get("phases", 6)
    nc = tc.nc
    npc, half, nb = meta["npc"], meta["half"], meta["nb"]
    npad = meta["npad"]
    T, tile_start = meta["T"], meta["tile_start"]
    tb_max = meta["tb_max"]

    t1_slice = nc.dram_tensor("t1_slice", [npc, ELEM], F32)
    t1_full = nc.dram_tensor("t1_full", [npad, ELEM], F32, addr_space="Shared")
    t2_slice = nc.dram_tensor("t2_slice", [npc, ELEM], F32)
    t2_full = nc.dram_tensor("t2_full", [npad, ELEM], F32, addr_space="Shared")
    av1_local = nc.dram_tensor("av1_local", [npc, AVW], F32)
    av2_local = nc.dram_tensor("av2_local", [npc, AVW], F32)

    with ExitStack() as ctx:
        consts = ctx.enter_context(tc.tile_pool(name="consts", bufs=1))
        stage = ctx.enter_context(tc.tile_pool(name="stage", bufs=2))
        idxp = ctx.enter_context(tc.tile_pool(name="idxp", bufs=2))
        gat = ctx.enter_context(tc.tile_pool(name="gat", bufs=2))
        adp = ctx.enter_context(tc.tile_pool(name="adp", bufs=2))
        dp = ctx.enter_context(tc.tile_pool(name="dp", bufs=2))
        e4p = ctx.enter_context(tc.tile_pool(name="e4p", bufs=2))
        zp = ctx.enter_context(tc.tile_pool(name="zp", bufs=2))
        zTp = ctx.enter_context(tc.tile_pool(name="zTp", bufs=1))
        pp = ctx.enter_context(tc.tile_pool(name="pp", bufs=2, space="PSUM"))

        # constants
        xT_t = consts.tile([IN_CH, npc], F32)
        nc.sync.dma_start(out=xT_t[:], in_=ins["xT"][:])
        w1_t = consts.tile([IN_CH, C + 8], F32)
        nc.sync.dma_start(out=w1_t[:], in_=ins["W1av"][:])
        w2a_t = consts.tile([P, C + 8], F32)
        nc.sync.dma_start(out=w2a_t[:], in_=ins["W2av0"][:])
        w2b_t = consts.tile([P, C + 8], F32)
        nc.sync.dma_start(out=w2b_t[:], in_=ins["W2av1"][:])
        wc_t = consts.tile([HID, OUT_CH], F32)
        nc.sync.dma_start(out=wc_t[:], in_=ins["Wc"][:])
        b1_t = consts.tile([P, C], F32)
        nc.sync.dma_start(out=b1_t[:], in_=ins["b1r"][:])
        b2_t = consts.tile([P, HID], F32)
        nc.sync.dma_start(out=b2_t[:], in_=ins["b2r"][:])
        bc_t = consts.tile([P, OUT_CH], F32)
        nc.sync.dma_start(out=bc_t[:], in_=ins["bcr"][:])
        ident = consts.tile([P, P], F32)
        make_identity(nc, ident[:])

        # pre-allocate gpsimd registers for gather counts (register pool is
        # small; to_reg per call exhausts it)
        _nreg = {}
        for b in range(nb):
            for v in (int(T[b, 0]) * P, int(T[b, 1]) * P,
                      (int(T[b, 0]) + int(T[b, 1])) * P):
                if v and v not in _nreg:
                    _nreg[v] = nc.gpsimd.to_reg(v)

        zT0 = zTp.tile([P, npc], F32, tag="zT0")
        zT1 = zTp.tile([P, npc], F32, tag="zT1")
        z2T = zTp.tile([HID, npc], F32, tag="z2T")

        def write_table(b, psum, tslice, avlocal):
            st = stage.tile([P, ELEM], F32, tag="stage")
            nc.vector.tensor_copy(st[:, 0:C + 8], psum[:])
            nc.vector.memset(st[:, C + 8:ELEM], 0.0)
            nc.sync.dma_start(out=tslice[b * P:(b + 1) * P, :], in_=st[:])
            nc.sync.dma_start(out=avlocal[b * P:(b + 1) * P, :], in_=st[:, C:C + AVW])

        # ---- P1: layer-1 tables: g1 = x @ W1 (+ folded alpha columns)
        for b in range(nb):
            psum = pp.tile([P, C + 8], F32, tag="mm")
            nc.tensor.matmul(psum[:], xT_t[:, b * P:(b + 1) * P], w1_t[:],
                             start=True, stop=True)
            write_table(b, psum, t1_slice, av1_local)

        if phases < 2:
            return
        # ---- P2: AllGather layer-1 table
        if not meta.get("skip_ag"):
            nc.gpsimd.collective_compute(
                "AllGather", mybir.AluOpType.bypass,
                replica_groups=[list(range(NC))],
                ins=[t1_slice[:]], outs=[t1_full[:]],
            )

        sub = meta.get("sub", 0)
        scratch = nc.dram_tensor("scratch_dbg", [P, 64], F32) if sub else None

        def edge_pass(table_full, av_local, post_fn, av_src23=None):
            for b in range(nb):
                ts0 = int(tile_start[b])
                t0, t1 = int(T[b, 0]), int(T[b, 1])
                tb = t0 + t1
                if tb == 0:
                    continue
                do_g1 = sub in (0, 1, 2, 3, 11, 14, 15)
                do_g3 = sub in (0, 1, 2, 3, 12, 14, 15, 22, 23) or (sub == 21 and b == 0)
                do_d = sub in (0, 1, 2, 3, 13, 14, 15)
                idx_t = idxp.tile([P, 8 * tb], I16, tag="sidx")
                nc.sync.dma_start(
                    out=idx_t[:], in_=ins["srcidx"][:, 8 * ts0: 8 * (ts0 + tb)])
                idx2_t = idxp.tile([P, 8 * tb], I16, tag="didx")
                nc.sync.dma_start(
                    out=idx2_t[:], in_=ins["dstidx"][:, 8 * ts0: 8 * (ts0 + tb)])

                g_t = gat.tile([P, tb_max, ELEM], F32, tag="gt")
                if t0 and do_g1:
                    nc.gpsimd.dma_gather(
                        out_ap=g_t[:, 0:t0, :],
                        in_ap=table_full[0:half, :],
                        idxs_ap=idx_t[:, 0:8 * t0],
                        num_idxs=t0 * P, num_idxs_reg=_nreg[t0 * P], elem_size=ELEM,
                        single_packet=(t0 * P <= 1024),
                    )
                if t1 and do_g1:
                    nc.gpsimd.dma_gather(
                        out_ap=g_t[:, t0:tb, :],
                        in_ap=table_full[half:npad, :],
                        idxs_ap=idx_t[:, 8 * t0:8 * tb],
                        num_idxs=t1 * P, num_idxs_reg=_nreg[t1 * P], elem_size=ELEM,
                        single_packet=(t1 * P <= 1024),
                    )
                ad_t = adp.tile([P, tb_max, AVW], F32, tag="ad")
                if do_g3 and sub == 23:
                    gd_t = gat.tile([P, tb_max, ELEM], F32, tag="gt23")
                    nc.gpsimd.dma_gather(
                        out_ap=gd_t[:, 0:tb, :],
                        in_ap=av_src23[:],
                        idxs_ap=idx2_t[:],
                        num_idxs=tb * P,
                        num_idxs_reg=_nreg[tb * P],
                        elem_size=ELEM,
                        single_packet=(tb * P <= 1024),
                    )
                    nc.vector.tensor_copy(ad_t[:, 0:tb, 0:8], gd_t[:, 0:tb, C:C + 8])
                elif do_g3:
                    nc.gpsimd.dma_gather(
                        out_ap=ad_t[:, 0:tb, :],
                        in_ap=av_local[:],
                        idxs_ap=idx2_t[:],
                        num_idxs=tb * P,
                        num_idxs_reg=(_nreg[tb * P] if sub not in (15, 22)
                                      else nc.gpsimd.to_reg(tb * P + 0)),
                        elem_size=AVW,
                        single_packet=(tb * P <= 1024),
                    )
                d_t = dp.tile([P, tb_max, P], F32, tag="dm")
                if do_d:
                    nc.sync.dma_start(
                        out=d_t[:, 0:tb, :],
                        in_=ins["Dmat"][ts0 * P:(ts0 + tb) * P, :]
                            .rearrange("(t p) n -> p t n", p=P),
                    )

                if sub in (1, 11, 12, 13, 14, 15, 21, 22, 23):
                    if do_g1:
                        nc.sync.dma_start(out=scratch[:, 0:ELEM//8],
                                          in_=g_t[:, 0, 0:ELEM:8])
                    if do_g3:
                        nc.sync.dma_start(out=scratch[:, 0:AVW], in_=ad_t[:, 0, :])
                    if do_d:
                        nc.sync.dma_start(out=scratch[:, 0:P//2], in_=d_t[:, 0, 0:P:2])
                    continue

                # e4 = exp(leakyrelu(a_src + a_dst)), written over the a_src cols
                e4 = g_t[:, 0:tb, C:C + 4]
                nc.vector.tensor_tensor(
                    out=e4, in0=e4, in1=ad_t[:, 0:tb, 4:8],
                    op=mybir.AluOpType.add)
                tmp4 = e4p.tile([P, tb_max, 4], F32, tag="t4")
                nc.vector.tensor_scalar_mul(tmp4[:, 0:tb], e4, NEG_SLOPE)
                nc.vector.tensor_tensor(
                    out=e4, in0=e4, in1=tmp4[:, 0:tb], op=mybir.AluOpType.max)
                nc.scalar.activation(e4, e4, mybir.ActivationFunctionType.Exp)

                # fold attention weights into gathered h rows (in place)
                nc.vector.tensor_tensor(
                    out=g_t[:, 0:tb, 0:C].rearrange("p t (h c) -> p t h c", h=HEADS),
                    in0=g_t[:, 0:tb, 0:C].rearrange("p t (h c) -> p t h c", h=HEADS),
                    in1=g_t[:, 0:tb, C:C + 4].unsqueeze(-1)
                        .to_broadcast([P, tb, HEADS, HID]),
                    op=mybir.AluOpType.mult)

                if sub == 2:
                    nc.sync.dma_start(out=scratch[:, 0:ELEM//8],
                                      in_=g_t[:, 0, 0:ELEM:8])
                    continue

                # scatter-accumulate: psum[n, 0:260] += D_t.T @ [m | e4]
                psum = pp.tile([P, C + 4], F32, tag="edge")
                for t in range(tb):
                    nc.tensor.matmul(
                        psum[:], d_t[:, t], g_t[:, t, 0:C + 4],
                        start=(t == 0), stop=(t == tb - 1))
                if sub == 3:
                    st3 = zp.tile([P, C + 4], F32, tag="dbg3")
                    nc.vector.tensor_copy(st3[:], psum[:])
                    nc.sync.dma_start(out=scratch[:, 0:C + 4:8], in_=st3[:, 0:C + 4:8])
                    continue
                post_fn(b, psum)

        def normalize(psum, out_ap):
            """out = psum[:, 0:C] / broadcast(psum[:, C:C+4])"""
            rden = e4p.tile([P, 4], F32, tag="rd")
            nc.vector.tensor_scalar_max(rden[:], psum[:, C:C + 4], 1e-30)
            nc.vector.reciprocal(rden[:], rden[:])
            nc.vector.tensor_tensor(
                out=out_ap.rearrange("p (h c) -> p h c", h=HEADS),
                in0=psum[:, 0:C].rearrange("p (h c) -> p h c", h=HEADS),
                in1=rden[:].unsqueeze(-1).to_broadcast([P, HEADS, HID]),
                op=mybir.AluOpType.mult)

        def elu_inplace(z, width, tag):
            """z = ELU(z) = (max(z,0) - 1) + exp(min(z,0))"""
            a = zp.tile([P, width], F32, tag=tag + "a")
            nc.vector.tensor_scalar_min(a[:], z, 0.0)
            nc.scalar.activation(a[:], a[:], mybir.ActivationFunctionType.Exp)
            d = zp.tile([P, width], F32, tag=tag + "d")
            nc.vector.tensor_scalar(
                out=d[:], in0=z, scalar1=0.0, scalar2=1.0,
                op0=mybir.AluOpType.max, op1=mybir.AluOpType.subtract)
            nc.vector.tensor_tensor(z, d[:], a[:], op=mybir.AluOpType.add)

        def post1(b, psum):
            z = zp.tile([P, C], F32, tag="z1")
            normalize(psum, z[:])
            nc.vector.tensor_tensor(z[:], z[:], b1_t[:], op=mybir.AluOpType.add)
            elu_inplace(z[:], C, "e1")
            for i, zT in enumerate((zT0, zT1)):
                pt = pp.tile([P, P], F32, tag="tp")
                nc.tensor.transpose(pt[:], z[:, i * P:(i + 1) * P], ident[:])
                nc.vector.tensor_copy(zT[:, b * P:(b + 1) * P], pt[:])

        def post2(b, psum):
            zn = zp.tile([P, C], F32, tag="z2n")
            normalize(psum, zn[:])
            hm = zp.tile([P, HID], F32, tag="hm")
            nc.vector.tensor_reduce(
                out=hm[:],
                in_=zn[:].rearrange("p (h c) -> p c h", h=HEADS),
                axis=mybir.AxisListType.X, op=mybir.AluOpType.add)
            nc.vector.tensor_scalar_mul(hm[:], hm[:], 1.0 / HEADS)
            nc.vector.tensor_tensor(hm[:], hm[:], b2_t[:], op=mybir.AluOpType.add)
            elu_inplace(hm[:], HID, "e2")
            pt = pp.tile([HID, P], F32, tag="tp")
            nc.tensor.transpose(pt[:], hm[:], ident[:])
            nc.vector.tensor_copy(z2T[:, b * P:(b + 1) * P], pt[:])

        # ---- P3: layer-1 message passing
        if phases < 3:
            return
        edge_pass(t1_full, av1_local, post1, av_src23=t1_slice)

        # ---- P4: layer-2 tables: g2 = z1 @ W2 (+ folded alpha columns)
        if phases < 4:
            return
        for b in range(nb):
            psum = pp.tile([P, C + 8], F32, tag="mm")
            nc.tensor.matmul(psum[:], zT0[:, b * P:(b + 1) * P], w2a_t[:],
                             start=True, stop=False)
            nc.tensor.matmul(psum[:], zT1[:, b * P:(b + 1) * P], w2b_t[:],
                             start=False, stop=True)
            write_table(b, psum, t2_slice, av2_local)

        if phases < 5:
            return
        # ---- P5: AllGather layer-2 table + message passing
        nc.gpsimd.collective_compute(
            "AllGather", mybir.AluOpType.bypass,
            replica_groups=[list(range(NC))],
            ins=[t2_slice[:]], outs=[t2_full[:]],
        )
        edge_pass(t2_full, av2_local, post2)

        # ---- P6: final projection y = z2 @ Wc + bc
        if phases < 6:
            return
        for b in range(nb):
            psum = pp.tile([P, OUT_CH], F32, tag="mm")
            nc.tensor.matmul(psum[:], z2T[:, b * P:(b + 1) * P], wc_t[:],
                             start=True, stop=True)
            yt = zp.tile([P, OUT_CH], F32, tag="yt")
            nc.vector.tensor_tensor(yt[:], psum[:], bc_t[:], op=mybir.AluOpType.add)
            nc.sync.dma_start(out=outs["y"][b * P:(b + 1) * P, :], in_=yt[:])


# ----------------------------------------------------------------------------
# entry point
# ----------------------------------------------------------------------------

def _prepare(inputs, n_nodes, npc):
    """Full host-side prep: edges, weights, per-core input maps."""
    ei = np.asarray(inputs["edge_index"])
    src = np.concatenate([ei[0], np.arange(n_nodes, dtype=ei.dtype)]).astype(np.int64)
    dst = np.concatenate([ei[1], np.arange(n_nodes, dtype=ei.dtype)]).astype(np.int64)
    meta, per_core = _prep_edges(src, dst, n_nodes, npc)
    npad = meta["npad"]

    x = np.asarray(inputs["x"], np.float32)
    xTp = np.zeros((IN_CH, npad), np.float32)
    xTp[:, :n_nodes] = x.T

    W1av = _fold_weights(np.asarray(inputs["W1"], np.float32),
                         np.asarray(inputs["as1"], np.float32),
                         np.asarray(inputs["ad1"], np.float32))
    W2av = _fold_weights(np.asarray(inputs["W2"], np.float32),
                         np.asarray(inputs["as2"], np.float32),
                         np.asarray(inputs["ad2"], np.float32))
    b1r = np.tile(np.asarray(inputs["b1"], np.float32)[None, :], (P, 1))
    b2r = np.tile(np.asarray(inputs["b2"], np.float32)[None, :], (P, 1))
    bcr = np.tile(np.asarray(inputs["bc"], np.float32)[None, :], (P, 1))
    Wc = np.asarray(inputs["Wc"], np.float32)

    in_maps = []
    for k in range(NC):
        m = {
            "xT": np.ascontiguousarray(xTp[:, k * npc:(k + 1) * npc]),
            "W1av": W1av,
            "W2av0": np.ascontiguousarray(W2av[0:P]),
            "W2av1": np.ascontiguousarray(W2av[P:C]),
            "Wc": Wc,
            "b1r": b1r, "b2r": b2r, "bcr": bcr,
            "srcidx": per_core[k]["srcidx"],
            "dstidx": per_core[k]["dstidx"],
            "Dmat": per_core[k]["Dmat"],
        }
        in_maps.append(m)
    return meta, in_maps


def _declare_and_build(nc, meta, sample_map):
    """Declare externals on nc and run the builder inside a TileContext."""
    ins = {}
    for name, arr in sample_map.items():
        ins[name] = nc.dram_tensor(
            name, list(arr.shape), mybir.dt.from_np(arr.dtype), kind="ExternalInput"
        ).ap()
    y = nc.dram_tensor("y", [meta["npc"], OUT_CH], F32, kind="ExternalOutput").ap()
    with tile.TileContext(nc) as tc:
        build_gat(tc, {"y": y}, ins, meta)
    nc.compile()


TRACE = False
LAST_RESULT = None


def kernel(**inputs) -> np.ndarray:
    global LAST_RESULT
    from concourse.bass_utils import run_bass_kernel_spmd

    n_nodes = inputs["x"].shape[0]
    npc = -(-n_nodes // (NC * P)) * P        # nodes per core, 128-aligned
    meta, in_maps = _prepare(inputs, n_nodes, npc)

    nc = bacc.Bacc("TRN2", target_bir_lowering=False)
    _declare_and_build(nc, meta, in_maps[0])

    res = run_bass_kernel_spmd(nc, in_maps, core_ids=list(range(NC)), trace=TRACE)
    LAST_RESULT = res
    y = np.concatenate([r["y"] for r in res.results], axis=0)[:n_nodes]
    return y.astype(np.float32)



# revision 21
# speedup vs baseline: 1.2223x; 1.2223x over previous
"""GAT (2-layer, PyG-style) Trainium2 Bass kernel, 8-core SPMD.

Destination-node partitioning: each core owns a contiguous range of dst nodes
and all edges into it (host pre-groups edges by (dst block, src half)).

Layer 1 gathers raw x rows (padded to 256B, bf16) with dma_gather
transpose=True so they arrive channel-partitioned, then expands h|a_src per
edge with a K=16 matmul against W1|W1@As (no node table, no AllGather).
Layer 2 builds a per-node table [h2|as2] (768B bf16 rows) during layer-1
post-processing, AllGathers it once, and edge-gathers rows directly.

Per edge tile (128 edges): a_dst arrives via a small DT@av matmul (DT = host
one-hot dst transpose), e4 = exp(leakyrelu(as+ad)) is fused into the message
multiply, and a device-generated one-hot D (iota is_equal dloc) turns
scatter-add into PSUM matmul accumulation with softmax denominators riding as
4 extra rhs columns. Gathers round-robin over 4 SWDGE queues so descriptor
generation runs on all four Q7 cpu pairs concurrently.
"""

from contextlib import ExitStack

import numpy as np
import ml_dtypes

import concourse.bass as bass
import concourse.bacc as bacc
import concourse.mybir as mybir
import concourse.tile as tile
from concourse.masks import make_identity

P = 128
NC = 8
IN_CH = 16
HEADS = 4
HID = 64
C = HEADS * HID          # 256
OUT_CH = 8
ELEM2 = 384              # L2 table row: h(256) | as(4) | pad -> 384 bf16 = 768 B
NEG_SLOPE = 0.2
SB = 2                   # dst blocks per gather call
NQ = 4                   # SWDGE queues
F32 = mybir.dt.float32
BF16 = mybir.dt.bfloat16
I16 = mybir.dt.int16

BF1 = np.uint16(0x3F80)  # 1.0 in bf16 bits


def _bf16(x):
    return np.asarray(x, ml_dtypes.bfloat16).view(np.uint16)


# ----------------------------------------------------------------------------
# host-side preprocessing
# ----------------------------------------------------------------------------

def _prep_edges(src, dst, npc):
    """Group edges per core by (dst block, src half); build shared tile meta
    plus per-core index/dloc/DT arrays."""
    npad = NC * npc
    half = npad // 2
    nb = npc // P
    assert npc % P == 0 and half <= 32768

    core_of = dst // npc
    per_core = []
    counts = np.zeros((NC, nb, 2), np.int64)
    for k in range(NC):
        sel = core_of == k
        s = src[sel]
        dl = dst[sel] - k * npc
        blk = dl >> 7
        hlf = s // half
        order = np.lexsort((s, hlf, blk))
        s, dl, blk, hlf = s[order], dl[order], blk[order], hlf[order]
        np.add.at(counts[k], (blk, hlf), 1)
        # group start offsets in sorted arrays
        gstart = np.zeros((nb, 2), np.int64)
        gcnt = np.zeros((nb, 2), np.int64)
        idx = 0
        for b in range(nb):
            for h in range(2):
                cnt = int(((blk == b) & (hlf == h)).sum())
                gstart[b, h] = idx
                gcnt[b, h] = cnt
                idx += cnt
        per_core.append((s, dl, gstart, gcnt))

    T = np.ceil(counts.max(axis=0) / P).astype(np.int64)   # [nb, 2]
    TT = int(T.sum())
    tb = T.sum(axis=1)                                     # tiles per block
    tb_max = int(tb.max())

    nsb = -(-nb // SB)
    # calls: (sb, h) -> col start (in 16-wrapped units), nidx
    calls = []
    cs = 0
    call_id = {}
    for sb in range(nsb):
        blocks = list(range(sb * SB, min((sb + 1) * SB, nb)))
        for h in range(2):
            nidx = int(sum(T[b, h] for b in blocks) * P)
            call_id[(sb, h)] = len(calls)
            calls.append({"sb": sb, "h": h, "cs": cs, "nidx": nidx,
                          "blocks": blocks})
            cs += nidx // 16
    CT = cs

    # processing order tiles: for sb, for b in sb, for h, for tile
    tiles = []           # (b, h, call, off_in_call, pt)
    pt = 0
    pt_start = np.zeros(nb + 1, np.int64)
    for sb in range(nsb):
        blocks = calls[call_id[(sb, 0)]]["blocks"]
        for bi, b in enumerate(blocks):
            pt_start[b] = pt
            for h in range(2):
                off = int(sum(T[bb, h] for bb in blocks[:bi]))
                for i in range(int(T[b, h])):
                    tiles.append((b, h, call_id[(sb, h)], off + i, pt))
                    pt += 1
    pt_start[nb] = pt
    assert pt == TT

    meta = {"npc": npc, "npad": npad, "half": half, "nb": nb, "nsb": nsb,
            "T": T, "TT": TT, "tb": tb, "tb_max": tb_max, "calls": calls,
            "tiles": tiles, "pt_start": pt_start, "CT": CT}

    per_core_arrays = []
    for k in range(NC):
        s, dl, gstart, gcnt = per_core[k]
        idx16 = np.zeros((16, CT), np.int16)
        dloc = np.full(TT * P, -1, np.int64)
        gpos = np.zeros((nb, 2), np.int64)   # consumed edges per group
        for (b, h, c, off, ptt) in tiles:
            call = calls[c]
            g0 = int(gstart[b, h]) + int(gpos[b, h])
            n = min(int(gcnt[b, h]) - int(gpos[b, h]), P)
            gpos[b, h] += n
            if n <= 0:
                continue
            sl = np.arange(n)
            j = off * P + sl                      # slot within call
            col = call["cs"] + j // 16
            idx16[j % 16, col] = (s[g0:g0 + n] % half).astype(np.int16)
            dloc[ptt * P + sl] = dl[g0:g0 + n] & 127

        DT = np.zeros((P, TT * P), np.uint16)
        valid = dloc >= 0
        DT[dloc[valid], np.where(valid)[0]] = BF1

        per_core_arrays.append({
            "srcidx": np.tile(idx16, (8, 1)),
            "dlocc": np.ascontiguousarray(
                dloc.reshape(TT, P).T.astype(np.float32)),
            "DT": DT,
        })
    return meta, per_core_arrays


def _fold_as(a_s):
    As = np.zeros((C, HEADS), np.float32)
    for h in range(HEADS):
        As[h * HID:(h + 1) * HID, h] = a_s[h]
    return As


# ----------------------------------------------------------------------------
# device program
# ----------------------------------------------------------------------------

def build_gat(tc, outs, ins, meta):
    nc = tc.nc
    npc, half, nb, nsb = meta["npc"], meta["half"], meta["nb"], meta["nsb"]
    npad = meta["npad"]
    T, calls, tiles = meta["T"], meta["calls"], meta["tiles"]
    tb, tb_max, TT = meta["tb"], meta["tb_max"], meta["TT"]
    pt_start = meta["pt_start"]
    phases = meta.get("phases", 4)

    t2_slice = nc.dram_tensor("t2_slice", [npc, ELEM2], BF16)
    t2_full = nc.dram_tensor("t2_full", [npad, ELEM2], BF16,
                             addr_space="Shared")
    dump = meta.get("dump")
    dbg = outs.get("dbg")

    with ExitStack() as ctx:
        consts = ctx.enter_context(tc.tile_pool(name="consts", bufs=1))
        gp = ctx.enter_context(tc.tile_pool(name="gp", bufs=4))
        dtp = ctx.enter_context(tc.tile_pool(name="dtp", bufs=2))
        dp = ctx.enter_context(tc.tile_pool(name="dp", bufs=2))
        mp = ctx.enter_context(tc.tile_pool(name="mp", bufs=2))
        zp = ctx.enter_context(tc.tile_pool(name="zp", bufs=2))
        e4p = ctx.enter_context(tc.tile_pool(name="e4p", bufs=2))
        pp = ctx.enter_context(tc.tile_pool(name="pp", bufs=1, space="PSUM"))

        # ---- constants
        idx_t = consts.tile([P, meta["CT"]], I16)
        nc.sync.dma_start(out=idx_t[:], in_=ins["srcidx"][:])
        dloc_t = consts.tile([P, TT], F32)
        nc.sync.dma_start(out=dloc_t[:], in_=ins["dlocc"][:])
        iota_t = consts.tile([P, P], F32)
        nc.sync.dma_start(out=iota_t[:], in_=ins["iota"][:])
        w1_t = consts.tile([IN_CH, C + 4], BF16)
        nc.sync.dma_start(out=w1_t[:], in_=ins["W1av"][:].bitcast(BF16))
        w1ad_t = consts.tile([IN_CH, HEADS], BF16)
        nc.sync.dma_start(out=w1ad_t[:], in_=ins["W1Ad"][:].bitcast(BF16))
        w2e_t = consts.tile([P, C + 8], BF16)
        nc.sync.dma_start(out=w2e_t[:], in_=ins["W2avdE"][:].bitcast(BF16))
        w2o_t = consts.tile([P, C + 8], BF16)
        nc.sync.dma_start(out=w2o_t[:], in_=ins["W2avdO"][:].bitcast(BF16))
        wce_t = consts.tile([HID // 2, OUT_CH], BF16)
        nc.sync.dma_start(out=wce_t[:], in_=ins["WcE"][:].bitcast(BF16))
        wco_t = consts.tile([HID // 2, OUT_CH], BF16)
        nc.sync.dma_start(out=wco_t[:], in_=ins["WcO"][:].bitcast(BF16))
        b1_t = consts.tile([P, C], F32)
        nc.sync.dma_start(out=b1_t[:], in_=ins["b1r"][:])
        b2_t = consts.tile([P, HID], F32)
        nc.sync.dma_start(out=b2_t[:], in_=ins["b2r"][:])
        bc_t = consts.tile([P, OUT_CH], F32)
        nc.sync.dma_start(out=bc_t[:], in_=ins["bcr"][:])
        ident = consts.tile([P, P], F32)
        make_identity(nc, ident[:])
        av1 = consts.tile([P, nb * HEADS], BF16)
        av2 = consts.tile([P, nb * HEADS], BF16)

        nregs = {}
        for c in calls:
            if c["nidx"] and c["nidx"] not in nregs:
                nregs[c["nidx"]] = nc.gpsimd.to_reg(c["nidx"])

        # ---- P-A: av1[n] = x[n] @ (W1@Ad1)  (block-local a_dst table)
        for b in range(nb):
            xTb = zp.tile([IN_CH, P], BF16, tag="xTb")
            nc.sync.dma_start(
                out=xTb[:], in_=ins["xTloc"][:, b * P:(b + 1) * P]
                    .bitcast(BF16))
            ps = pp.tile([P, C + 4], F32, tag="e", bufs=2)
            nc.tensor.matmul(ps[:, 0:HEADS], xTb[:], w1ad_t[:],
                             start=True, stop=True)
            nc.vector.tensor_copy(av1[:, b * HEADS:(b + 1) * HEADS],
                                  ps[:, 0:HEADS])

        if phases < 2:
            return

        qrr = [0]

        def gather(c, layer):
            if layer == 1:
                q = 0          # transpose gathers share the xbar; serialize
            else:
                q = qrr[0] % NQ
                qrr[0] += 1
            nidx = c["nidx"]
            if nidx == 0:
                return None
            cs = c["cs"]
            h = c["h"]
            if layer == 1:
                g = gp.tile([P, 1, nidx], BF16, tag=f"g1{h}", bufs=4)
                nc.gpsimd.dma_gather(
                    out_ap=g[:], in_ap=ins["xtab"][h * half:(h + 1) * half, :]
                        .bitcast(BF16),
                    idxs_ap=idx_t[:, cs:cs + nidx // 16],
                    num_idxs=nidx, num_idxs_reg=nregs[nidx],
                    elem_size=P, transpose=True, single_packet=False,
                    queue_num=q)
            else:
                g = gp.tile([P, nidx // P, ELEM2], BF16, tag=f"g2{h}", bufs=2)
                nc.gpsimd.dma_gather(
                    out_ap=g[:], in_ap=t2_full[h * half:(h + 1) * half, :],
                    idxs_ap=idx_t[:, cs:cs + nidx // 16],
                    num_idxs=nidx, num_idxs_reg=nregs[nidx],
                    elem_size=ELEM2, transpose=False, single_packet=False,
                    queue_num=q)
            return g

        def edge_pass(layer, post_fn):
            ti = 0
            for sb in range(nsb):
                c0 = calls[2 * sb]
                c1 = calls[2 * sb + 1]
                g0 = gather(c0, layer)
                g1 = gather(c1, layer)
                gs = (g0, g1)
                for b in c0["blocks"]:
                    ntile = int(tb[b])
                    if ntile == 0:
                        post_fn(b, None)
                        continue
                    dt_t = dtp.tile([P, tb_max * P], BF16, tag="dt")
                    p0 = int(pt_start[b])
                    nc.sync.dma_start(
                        out=dt_t[:, 0:ntile * P],
                        in_=ins["DT"][:, p0 * P:(p0 + ntile) * P]
                            .bitcast(BF16))
                    d_blk = dp.tile([P, tb_max, P], BF16, tag="d")
                    m_blk = mp.tile([P, tb_max, C + 4], BF16, tag="m")
                    for lt in range(ntile):
                        bb, h, ci, off, ptt = tiles[ti]
                        assert bb == b and ptt == p0 + lt
                        ti += 1
                        g = gs[h]
                        if layer == 1:
                            # expansion + ad[dst] fused into one psum:
                            # cols C:C+4 get as[src] + ad[dst] by accumulation
                            pse = pp.tile([P, C + 4], F32, tag="e", bufs=2)
                            nc.tensor.matmul(
                                pse[:, 0:C + 4],
                                g[0:IN_CH, 0, off * P:(off + 1) * P],
                                w1_t[:], start=True, stop=False)
                            nc.tensor.matmul(
                                pse[:, C:C + 4], dt_t[:, lt * P:(lt + 1) * P],
                                av1[:, b * HEADS:(b + 1) * HEADS],
                                start=False, stop=True)
                            h_src = pse[:, 0:C]
                            sum4 = pse[:, C:C + 4]
                        else:
                            h_src = g[:, off, 0:C]
                            psad = pp.tile([P, HEADS], F32, tag="ad", bufs=2)
                            nc.tensor.matmul(
                                psad[:], dt_t[:, lt * P:(lt + 1) * P],
                                av2[:, b * HEADS:(b + 1) * HEADS],
                                start=True, stop=True)
                            t4s = e4p.tile([P, HEADS], F32, tag="t4s")
                            nc.vector.tensor_tensor(
                                t4s[:], g[:, off, C:C + 4], psad[:],
                                op=mybir.AluOpType.add)
                            sum4 = t4s[:]
                        if dump and dump.startswith("pse") and b == 0 \
                                and lt == int(dump[3:] or 0) and layer == 1:
                            dt_dbg = zp.tile([P, C + 4], F32, tag="dbg")
                            nc.vector.tensor_copy(dt_dbg[:], pse[:])
                            nc.sync.dma_start(out=dbg[:, 0:C + 4],
                                              in_=dt_dbg[:])
                        # e4 = exp(leakyrelu(sum4)) -> m[:, lt, C:C+4]
                        t4b = e4p.tile([P, HEADS], F32, tag="t4b")
                        nc.vector.tensor_scalar_mul(t4b[:], sum4, NEG_SLOPE)
                        t4a = e4p.tile([P, HEADS], F32, tag="t4a")
                        nc.vector.tensor_tensor(
                            t4a[:], sum4, t4b[:], op=mybir.AluOpType.max)
                        nc.scalar.activation(
                            m_blk[:, lt, C:C + 4], t4a[:],
                            mybir.ActivationFunctionType.Exp)
                        # m = h_src * e4 (broadcast over HID)
                        nc.vector.tensor_tensor(
                            out=m_blk[:, lt, 0:C]
                                .rearrange("p (h c) -> p h c", h=HEADS),
                            in0=h_src.rearrange("p (h c) -> p h c", h=HEADS),
                            in1=m_blk[:, lt, C:C + 4].unsqueeze(-1)
                                .to_broadcast([P, HEADS, HID]),
                            op=mybir.AluOpType.mult)
                        # D one-hot [e, n]
                        nc.vector.tensor_scalar(
                            out=d_blk[:, lt], in0=iota_t[:],
                            scalar1=dloc_t[:, ptt:ptt + 1], scalar2=None,
                            op0=mybir.AluOpType.is_equal)
                    if dump and dump.startswith("dm") and b == 0 \
                            and layer == 1:
                        dlt = int(dump[2:])
                        dt_dbg4 = zp.tile([P, P + C + 4], F32, tag="dbg4")
                        nc.vector.tensor_copy(dt_dbg4[:, 0:P], d_blk[:, dlt])
                        nc.vector.tensor_copy(dt_dbg4[:, P:P + C + 4],
                                              m_blk[:, dlt])
                        nc.sync.dma_start(out=dbg[:, 0:P + C + 4],
                                          in_=dt_dbg4[:])
                    # scatter-accumulate: closed matmul groups + vector adds
                    psb = zp.tile([P, C + 4], F32, tag="acc")
                    for lt in range(ntile):
                        pst = pp.tile([P, C + 4], F32, tag="blk", bufs=2)
                        nc.tensor.matmul(
                            pst[:], d_blk[:, lt], m_blk[:, lt, 0:C + 4],
                            start=True, stop=True)
                        if lt == 0:
                            nc.vector.tensor_copy(psb[:], pst[:])
                        else:
                            nc.vector.tensor_tensor(
                                psb[:], psb[:], pst[:],
                                op=mybir.AluOpType.add)
                    if dump == "psb" and b == 0 and layer == 1:
                        dt_dbg3 = zp.tile([P, C + 4], F32, tag="dbg3")
                        nc.vector.tensor_copy(dt_dbg3[:], psb[:])
                        nc.sync.dma_start(out=dbg[:, 0:C + 4], in_=dt_dbg3[:])
                    post_fn(b, psb)

        def normalize(psum, out_ap):
            rden = e4p.tile([P, HEADS], F32, tag="rd")
            nc.vector.tensor_scalar_max(rden[:], psum[:, C:C + 4], 1e-30)
            nc.vector.reciprocal(rden[:], rden[:])
            nc.vector.tensor_tensor(
                out=out_ap.rearrange("p (h c) -> p h c", h=HEADS),
                in0=psum[:, 0:C].rearrange("p (h c) -> p h c", h=HEADS),
                in1=rden[:].unsqueeze(-1).to_broadcast([P, HEADS, HID]),
                op=mybir.AluOpType.mult)

        def elu_inplace(z, width, tag):
            a = zp.tile([P, width], F32, tag=tag + "a")
            nc.vector.tensor_scalar_min(a[:], z, 0.0)
            nc.scalar.activation(a[:], a[:], mybir.ActivationFunctionType.Exp)
            d = zp.tile([P, width], F32, tag=tag + "d")
            nc.vector.tensor_scalar(
                out=d[:], in0=z, scalar1=0.0, scalar2=1.0,
                op0=mybir.AluOpType.max, op1=mybir.AluOpType.subtract)
            nc.vector.tensor_tensor(z, d[:], a[:], op=mybir.AluOpType.add)

        def pair_transpose(zb_bf16_asf32, rows):
            """[128, rows] f32(bf16-pairs) -> psum [rows, 128] transposed."""
            pt = pp.tile([P, P], F32, tag="tp")
            nc.tensor.transpose(pt[0:rows, :], zb_bf16_asf32, ident[:])
            return pt

        def post1(b, psb):
            z = zp.tile([P, C], F32, tag="z1")
            if psb is None:
                nc.vector.memset(z[:], 0.0)
            else:
                normalize(psb, z[:])
            nc.vector.tensor_tensor(z[:], z[:], b1_t[:], op=mybir.AluOpType.add)
            elu_inplace(z[:], C, "e1")
            if dump == "z1" and b == 0:
                nc.sync.dma_start(out=dbg[:, 0:C], in_=z[:])
            zb = zp.tile([P, C], BF16, tag="zb1")
            nc.vector.tensor_copy(zb[:], z[:])
            ptp = pair_transpose(zb[:].bitcast(F32), P)
            zT4 = zp.tile([P, P], F32, tag="zT4")
            nc.vector.tensor_copy(zT4[:], ptp[:])
            zT4b = zT4[:].bitcast(BF16).rearrange("p (n two) -> p n two", two=2)
            ps2 = pp.tile([P, C + 8], F32, tag="t2b")
            nc.tensor.matmul(ps2[:], zT4b[:, :, 0], w2e_t[:],
                             start=True, stop=False)
            nc.tensor.matmul(ps2[:], zT4b[:, :, 1], w2o_t[:],
                             start=False, stop=True)
            st2 = zp.tile([P, C + 4], BF16, tag="st2")
            nc.vector.tensor_copy(st2[:], ps2[:, 0:C + 4])
            nc.vector.tensor_copy(av2[:, b * HEADS:(b + 1) * HEADS],
                                  ps2[:, C + 4:C + 8])
            nc.sync.dma_start(out=t2_slice[b * P:(b + 1) * P, 0:C + 4],
                              in_=st2[:])

        def post2(b, psb):
            zn = zp.tile([P, C], F32, tag="z2n")
            if psb is None:
                nc.vector.memset(zn[:], 0.0)
            else:
                normalize(psb, zn[:])
            hm = zp.tile([P, HID], F32, tag="hm")
            nc.vector.tensor_reduce(
                out=hm[:],
                in_=zn[:].rearrange("p (h c) -> p c h", h=HEADS),
                axis=mybir.AxisListType.X, op=mybir.AluOpType.add)
            nc.vector.tensor_scalar_mul(hm[:], hm[:], 1.0 / HEADS)
            nc.vector.tensor_tensor(hm[:], hm[:], b2_t[:],
                                    op=mybir.AluOpType.add)
            elu_inplace(hm[:], HID, "e2")
            hb = zp.tile([P, HID], BF16, tag="hb2")
            nc.vector.tensor_copy(hb[:], hm[:])
            ptp = pair_transpose(hb[:].bitcast(F32), HID // 2)
            zT2 = zp.tile([HID // 2, P], F32, tag="zT2")
            nc.vector.tensor_copy(zT2[:], ptp[0:HID // 2, :])
            zT2b = zT2[:].bitcast(BF16).rearrange("p (n two) -> p n two", two=2)
            psy = pp.tile([P, OUT_CH], F32, tag="ad", bufs=2)
            nc.tensor.matmul(psy[:], zT2b[:, :, 0], wce_t[:],
                             start=True, stop=False)
            nc.tensor.matmul(psy[:], zT2b[:, :, 1], wco_t[:],
                             start=False, stop=True)
            yt = zp.tile([P, OUT_CH], F32, tag="yt")
            nc.vector.tensor_tensor(yt[:], psy[:], bc_t[:],
                                    op=mybir.AluOpType.add)
            nc.sync.dma_start(out=outs["y"][b * P:(b + 1) * P, :], in_=yt[:])

        # ---- P-B: layer-1 edge pass (builds t2_slice and av2 in post1)
        edge_pass(1, post1)

        if phases < 3:
            return
        # ---- P-C: AllGather layer-2 table
        nc.gpsimd.collective_compute(
            "AllGather", mybir.AluOpType.bypass,
            replica_groups=[list(range(NC))],
            ins=[t2_slice[:]], outs=[t2_full[:]],
        )

        if phases < 4:
            return
        # ---- P-D: layer-2 edge pass
        edge_pass(2, post2)


# ----------------------------------------------------------------------------
# entry point
# ----------------------------------------------------------------------------

def _prepare(inputs, n_nodes, npc):
    ei = np.asarray(inputs["edge_index"])
    src = np.concatenate([ei[0], np.arange(n_nodes, dtype=ei.dtype)])
    src = src.astype(np.int64)
    dst = np.concatenate([ei[1], np.arange(n_nodes, dtype=ei.dtype)])
    dst = dst.astype(np.int64)
    meta, per_core = _prep_edges(src, dst, npc)
    npad = meta["npad"]

    x = np.asarray(inputs["x"], np.float32)
    xtab = np.zeros((npad, P), np.float32)
    xtab[:n_nodes, 0:IN_CH] = x
    xtab = _bf16(xtab)
    xT = np.zeros((IN_CH, npad), np.float32)
    xT[:, :n_nodes] = x.T
    xT = _bf16(xT)

    W1 = np.asarray(inputs["W1"], np.float32)
    W2 = np.asarray(inputs["W2"], np.float32)
    W1av = _bf16(np.concatenate(
        [W1, W1 @ _fold_as(np.asarray(inputs["as1"], np.float32))], axis=1))
    W1Ad = _bf16(W1 @ _fold_as(np.asarray(inputs["ad1"], np.float32)))
    W2avd = np.concatenate(
        [W2, W2 @ _fold_as(np.asarray(inputs["as2"], np.float32)),
         W2 @ _fold_as(np.asarray(inputs["ad2"], np.float32))], axis=1)
    W2avdE = _bf16(W2avd[0::2])
    W2avdO = _bf16(W2avd[1::2])
    Wc = np.asarray(inputs["Wc"], np.float32)
    b1r = np.tile(np.asarray(inputs["b1"], np.float32)[None, :], (P, 1))
    b2r = np.tile(np.asarray(inputs["b2"], np.float32)[None, :], (P, 1))
    bcr = np.tile(np.asarray(inputs["bc"], np.float32)[None, :], (P, 1))
    iota = np.tile(np.arange(P, dtype=np.float32)[None, :], (P, 1))

    in_maps = []
    for k in range(NC):
        m = {
            "xtab": xtab,
            "xTloc": np.ascontiguousarray(xT[:, k * npc:(k + 1) * npc]),
            "W1av": W1av, "W1Ad": W1Ad,
            "W2avdE": W2avdE, "W2avdO": W2avdO,
            "WcE": _bf16(Wc[0::2]), "WcO": _bf16(Wc[1::2]),
            "b1r": b1r, "b2r": b2r, "bcr": bcr, "iota": iota,
            "srcidx": per_core[k]["srcidx"],
            "dlocc": per_core[k]["dlocc"],
            "DT": per_core[k]["DT"],
        }
        in_maps.append(m)
    return meta, in_maps


def _declare_and_build(nc, meta, sample_map):
    ins = {}
    for name, arr in sample_map.items():
        ins[name] = nc.dram_tensor(
            name, list(arr.shape), mybir.dt.from_np(arr.dtype),
            kind="ExternalInput").ap()
    y = nc.dram_tensor("y", [meta["npc"], OUT_CH], F32, kind="ExternalOutput")
    outs = {"y": y.ap()}
    if meta.get("dump"):
        dbg = nc.dram_tensor("dbg", [P, 512], F32, kind="ExternalOutput")
        outs["dbg"] = dbg.ap()
    with tile.TileContext(nc) as tc:
        build_gat(tc, outs, ins, meta)
    nc.compile()


TRACE = False
LAST_RESULT = None
PHASES = 4
DUMP = None
CORES = NC


def kernel(**inputs) -> np.ndarray:
    global LAST_RESULT
    from concourse.bass_utils import run_bass_kernel_spmd

    n_nodes = inputs["x"].shape[0]
    npc = -(-n_nodes // (NC * P)) * P
    meta, in_maps = _prepare(inputs, n_nodes, npc)
    meta["phases"] = PHASES
    meta["dump"] = DUMP

    nc = bacc.Bacc("TRN2", target_bir_lowering=False, num_swdge_queues=NQ)
    _declare_and_build(nc, meta, in_maps[0])

    res = run_bass_kernel_spmd(nc, in_maps[:CORES], core_ids=list(range(CORES)),
                               trace=TRACE)
    LAST_RESULT = res
    y = np.concatenate([r["y"] for r in res.results], axis=0)[:n_nodes]
    return y.astype(np.float32)


# revision 29
# speedup vs baseline: 2.0000x; 1.6362x over previous
"""GAT (2-layer, PyG-style) Trainium2 Bass kernel, 8-core SPMD.

Destination-node partitioning: each core owns a contiguous range of dst nodes
and all edges into it (host pre-groups edges by (dst block, src half)).

Layer 1 gathers raw x rows (padded to 256B, bf16) with dma_gather
transpose=True so they arrive channel-partitioned, then expands h|a_src per
edge with a K=16 matmul against W1|W1@As (no node table, no AllGather).
Layer 2 builds a per-node table [h2|as2] (768B bf16 rows) during layer-1
post-processing, AllGathers it once, and edge-gathers rows directly.

Per edge tile (128 edges): a_dst arrives via a small DT@av matmul (DT = host
one-hot dst transpose), e4 = exp(leakyrelu(as+ad)) is fused into the message
multiply, and a device-generated one-hot D (iota is_equal dloc) turns
scatter-add into PSUM matmul accumulation with softmax denominators riding as
4 extra rhs columns. Gathers round-robin over 4 SWDGE queues so descriptor
generation runs on all four Q7 cpu pairs concurrently.
"""

from contextlib import ExitStack

import numpy as np
import ml_dtypes

import concourse.bass as bass
import concourse.bacc as bacc
import concourse.mybir as mybir
import concourse.tile as tile
from concourse.masks import make_identity

P = 128
NC = 8
IN_CH = 16
HEADS = 4
HID = 64
C = HEADS * HID          # 256
OUT_CH = 8
ELEM2 = 384              # L2 table row: h(256) | as(4) | pad -> 384 bf16 = 768 B
NEG_SLOPE = 0.2
SB = 2                   # dst blocks per gather call
NQ = 4                   # SWDGE queues
F32 = mybir.dt.float32
BF16 = mybir.dt.bfloat16
I16 = mybir.dt.int16

BF1 = np.uint16(0x3F80)  # 1.0 in bf16 bits


def _bf16(x):
    return np.asarray(x, ml_dtypes.bfloat16).view(np.uint16)


# ----------------------------------------------------------------------------
# host-side preprocessing
# ----------------------------------------------------------------------------

def _prep_edges(src, dst, npc):
    """Group edges per core by (dst block, src half); build shared tile meta
    plus per-core index/dloc/DT arrays."""
    npad = NC * npc
    half = npad // 2
    nb = npc // P
    assert npc % P == 0 and half <= 32768

    core_of = dst // npc
    per_core = []
    counts = np.zeros((NC, nb, 2), np.int64)
    for k in range(NC):
        sel = core_of == k
        s = src[sel]
        dl = dst[sel] - k * npc
        blk = dl >> 7
        hlf = s // half
        order = np.lexsort((s, hlf, blk))
        s, dl, blk, hlf = s[order], dl[order], blk[order], hlf[order]
        np.add.at(counts[k], (blk, hlf), 1)
        # group start offsets in sorted arrays
        gstart = np.zeros((nb, 2), np.int64)
        gcnt = np.zeros((nb, 2), np.int64)
        idx = 0
        for b in range(nb):
            for h in range(2):
                cnt = int(((blk == b) & (hlf == h)).sum())
                gstart[b, h] = idx
                gcnt[b, h] = cnt
                idx += cnt
        per_core.append((s, dl, gstart, gcnt))

    T = np.ceil(counts.max(axis=0) / P).astype(np.int64)   # [nb, 2]
    TT = int(T.sum())
    tb = T.sum(axis=1)                                     # tiles per block
    tb_max = int(tb.max())

    nsb = -(-nb // SB)
    # calls: (sb, h) -> col start (in 16-wrapped units), nidx
    calls = []
    cs = 0
    call_id = {}
    for sb in range(nsb):
        blocks = list(range(sb * SB, min((sb + 1) * SB, nb)))
        for h in range(2):
            nidx = int(sum(T[b, h] for b in blocks) * P)
            call_id[(sb, h)] = len(calls)
            calls.append({"sb": sb, "h": h, "cs": cs, "nidx": nidx,
                          "blocks": blocks})
            cs += nidx // 16
    CT = cs

    # processing order tiles: for sb, for b in sb, for h, for tile
    tiles = []           # (b, h, call, off_in_call, pt)
    pt = 0
    pt_start = np.zeros(nb + 1, np.int64)
    for sb in range(nsb):
        blocks = calls[call_id[(sb, 0)]]["blocks"]
        for bi, b in enumerate(blocks):
            pt_start[b] = pt
            for h in range(2):
                off = int(sum(T[bb, h] for bb in blocks[:bi]))
                for i in range(int(T[b, h])):
                    tiles.append((b, h, call_id[(sb, h)], off + i, pt))
                    pt += 1
    pt_start[nb] = pt
    assert pt == TT

    meta = {"npc": npc, "npad": npad, "half": half, "nb": nb, "nsb": nsb,
            "T": T, "TT": TT, "tb": tb, "tb_max": tb_max, "calls": calls,
            "tiles": tiles, "pt_start": pt_start, "CT": CT}

    per_core_arrays = []
    for k in range(NC):
        s, dl, gstart, gcnt = per_core[k]
        idx16 = np.zeros((16, CT), np.int16)
        dloc = np.full(TT * P, -1, np.int64)
        gpos = np.zeros((nb, 2), np.int64)   # consumed edges per group
        for (b, h, c, off, ptt) in tiles:
            call = calls[c]
            g0 = int(gstart[b, h]) + int(gpos[b, h])
            n = min(int(gcnt[b, h]) - int(gpos[b, h]), P)
            gpos[b, h] += n
            if n <= 0:
                continue
            sl = np.arange(n)
            j = off * P + sl                      # slot within call
            col = call["cs"] + j // 16
            idx16[j % 16, col] = (s[g0:g0 + n] % half).astype(np.int16)
            dloc[ptt * P + sl] = dl[g0:g0 + n] & 127

        DT = np.zeros((P, TT * P), np.uint16)
        valid = dloc >= 0
        vs = np.where(valid)[0]
        DT[dloc[valid], vs] = BF1
        Dm = np.zeros((P, TT * P), np.uint16)
        Dm[vs % P, (vs // P) * P + dloc[valid]] = BF1

        per_core_arrays.append({
            "srcidx": np.tile(idx16, (8, 1)),
            "dlocc": np.ascontiguousarray(
                dloc.reshape(TT, P).T.astype(np.float32)),
            "DT": DT,
            "Dm": Dm,
        })
    return meta, per_core_arrays


def _fold_as(a_s):
    As = np.zeros((C, HEADS), np.float32)
    for h in range(HEADS):
        As[h * HID:(h + 1) * HID, h] = a_s[h]
    return As


# ----------------------------------------------------------------------------
# device program
# ----------------------------------------------------------------------------

def build_gat(tc, outs, ins, meta):
    nc = tc.nc
    npc, half, nb, nsb = meta["npc"], meta["half"], meta["nb"], meta["nsb"]
    npad = meta["npad"]
    T, calls, tiles = meta["T"], meta["calls"], meta["tiles"]
    tb, tb_max, TT = meta["tb"], meta["tb_max"], meta["TT"]
    pt_start = meta["pt_start"]
    phases = meta.get("phases", 4)

    t2_slice = nc.dram_tensor("t2_slice", [npc, ELEM2], BF16)
    t2_full = nc.dram_tensor("t2_full", [npad, ELEM2], BF16,
                             addr_space="Shared")
    dump = meta.get("dump")
    dbg = outs.get("dbg")

    with ExitStack() as ctx:
        consts = ctx.enter_context(tc.tile_pool(name="consts", bufs=1))
        gp = ctx.enter_context(tc.tile_pool(name="gp", bufs=4))
        dtp = ctx.enter_context(tc.tile_pool(name="dtp", bufs=2))
        dp = ctx.enter_context(tc.tile_pool(name="dp", bufs=2))
        mp = ctx.enter_context(tc.tile_pool(name="mp", bufs=2))
        zp = ctx.enter_context(tc.tile_pool(name="zp", bufs=2))
        e4p = ctx.enter_context(tc.tile_pool(name="e4p", bufs=2))
        pp = ctx.enter_context(tc.tile_pool(name="pp", bufs=1, space="PSUM"))

        # ---- constants
        idx_t = consts.tile([P, meta["CT"]], I16)
        nc.sync.dma_start(out=idx_t[:], in_=ins["srcidx"][:])
        w1h_t = consts.tile([IN_CH, C], BF16)
        nc.sync.dma_start(out=w1h_t[:], in_=ins["W1av"][:, 0:C].bitcast(BF16))
        w1as_t = consts.tile([IN_CH, HEADS], BF16)
        nc.sync.dma_start(out=w1as_t[:],
                          in_=ins["W1av"][:, C:C + 4].bitcast(BF16))
        w1ad_t = consts.tile([IN_CH, HEADS], BF16)
        nc.sync.dma_start(out=w1ad_t[:], in_=ins["W1Ad"][:].bitcast(BF16))
        w2e_t = consts.tile([P, C + 8], BF16)
        nc.sync.dma_start(out=w2e_t[:], in_=ins["W2avdE"][:].bitcast(BF16))
        w2o_t = consts.tile([P, C + 8], BF16)
        nc.sync.dma_start(out=w2o_t[:], in_=ins["W2avdO"][:].bitcast(BF16))
        wce_t = consts.tile([HID // 2, OUT_CH], BF16)
        nc.sync.dma_start(out=wce_t[:], in_=ins["WcE"][:].bitcast(BF16))
        wco_t = consts.tile([HID // 2, OUT_CH], BF16)
        nc.sync.dma_start(out=wco_t[:], in_=ins["WcO"][:].bitcast(BF16))
        b1_t = consts.tile([P, C], F32)
        nc.sync.dma_start(out=b1_t[:], in_=ins["b1r"][:])
        b2_t = consts.tile([P, HID], F32)
        nc.sync.dma_start(out=b2_t[:], in_=ins["b2r"][:])
        bc_t = consts.tile([P, OUT_CH], F32)
        nc.sync.dma_start(out=bc_t[:], in_=ins["bcr"][:])
        ident = consts.tile([P, P], F32)
        make_identity(nc, ident[:])
        av1 = consts.tile([P, nb * HEADS], BF16)
        av2 = consts.tile([P, nb * HEADS], BF16)

        nregs = {}
        for c in calls:
            if c["nidx"] and c["nidx"] not in nregs:
                nregs[c["nidx"]] = nc.gpsimd.to_reg(c["nidx"])

        # ---- P-A: av1[n] = x[n] @ (W1@Ad1)  (block-local a_dst table)
        for b in range(nb):
            xTb = zp.tile([IN_CH, P], BF16, tag="xTb")
            nc.sync.dma_start(
                out=xTb[:], in_=ins["xTloc"][:, b * P:(b + 1) * P]
                    .bitcast(BF16))
            ps = pp.tile([P, C], F32, tag="e", bufs=2)
            nc.tensor.matmul(ps[:, 0:HEADS], xTb[:], w1ad_t[:],
                             start=True, stop=True)
            nc.vector.tensor_copy(av1[:, b * HEADS:(b + 1) * HEADS],
                                  ps[:, 0:HEADS])

        if phases < 2:
            return

        qrr = [0]

        def gather(c, layer):
            if layer == 1:
                q = 0          # transpose gathers share the xbar; serialize
            else:
                q = qrr[0] % NQ
                qrr[0] += 1
            nidx = c["nidx"]
            if nidx == 0:
                return None
            cs = c["cs"]
            h = c["h"]
            if layer == 1:
                g = gp.tile([P, 1, nidx], BF16, tag=f"g1{h}", bufs=4)
                nc.gpsimd.dma_gather(
                    out_ap=g[:], in_ap=ins["xtab"][h * half:(h + 1) * half, :]
                        .bitcast(BF16),
                    idxs_ap=idx_t[:, cs:cs + nidx // 16],
                    num_idxs=nidx, num_idxs_reg=nregs[nidx],
                    elem_size=P, transpose=True, single_packet=False,
                    queue_num=q)
            else:
                g = gp.tile([P, nidx // P, ELEM2], BF16, tag=f"g2{h}", bufs=2)
                nc.gpsimd.dma_gather(
                    out_ap=g[:], in_ap=t2_full[h * half:(h + 1) * half, :],
                    idxs_ap=idx_t[:, cs:cs + nidx // 16],
                    num_idxs=nidx, num_idxs_reg=nregs[nidx],
                    elem_size=ELEM2, transpose=False, single_packet=False,
                    queue_num=q)
            return g

        def edge_pass(layer, post_fn):
            ti = 0
            for sb in range(nsb):
                c0 = calls[2 * sb]
                c1 = calls[2 * sb + 1]
                g0 = gather(c0, layer)
                g1 = gather(c1, layer)
                gs = (g0, g1)
                for bi, b in enumerate(c0["blocks"]):
                    ntile = int(tb[b])
                    if ntile == 0:
                        post_fn(b, None)
                        continue
                    p0 = int(pt_start[b])
                    t0 = int(T[b, 0])
                    t1 = int(T[b, 1])
                    off_base = [int(sum(T[bb, hh] for bb in
                                        c0["blocks"][:bi])) for hh in (0, 1)]
                    dt_t = dtp.tile([P, tb_max * P], BF16, tag="dt")
                    nc.sync.dma_start(
                        out=dt_t[:, 0:ntile * P],
                        in_=ins["DT"][:, p0 * P:(p0 + ntile) * P]
                            .bitcast(BF16))
                    d_blk = dp.tile([P, tb_max, P], BF16, tag="d")
                    nc.sync.dma_start(
                        out=d_blk[:, 0:ntile, :],
                        in_=ins["Dm"][:, p0 * P:(p0 + ntile) * P]
                            .bitcast(BF16))
                    m_blk = mp.tile([P, tb_max, C + 4], BF16, tag="m")
                    s4 = pp.tile([P, tb_max * HEADS], F32, tag="s4", bufs=2)
                    avs = (av1 if layer == 1 else av2)[
                        :, b * HEADS:(b + 1) * HEADS]
                    for lt in range(ntile):
                        bb, h, ci, off, ptt = tiles[ti]
                        assert bb == b and ptt == p0 + lt
                        ti += 1
                        g = gs[h]
                        r4 = s4[:, lt * HEADS:(lt + 1) * HEADS]
                        dts = dt_t[:, lt * P:(lt + 1) * P]
                        if layer == 1:
                            xg = g[0:IN_CH, 0, off * P:(off + 1) * P]
                            pse = pp.tile([P, C], F32, tag="e", bufs=2)
                            nc.tensor.matmul(pse[:], xg, w1h_t[:],
                                             start=True, stop=True)
                            # as[src] then += ad[dst], one region of bank s4
                            nc.tensor.matmul(r4, xg, w1as_t[:],
                                             start=(lt == 0), stop=False)
                            nc.tensor.matmul(r4, dts, avs, start=False,
                                             stop=(lt == ntile - 1))
                            # evict h to m via the scalar engine (bf16 cast)
                            nc.scalar.activation(
                                m_blk[:, lt, 0:C], pse[:],
                                mybir.ActivationFunctionType.Copy)
                        else:
                            nc.tensor.matmul(r4, dts, avs, start=(lt == 0),
                                             stop=(lt == ntile - 1))
                    # ---- batched per-block e4 + fold
                    n4 = ntile * HEADS
                    s4v = s4[:, 0:n4]
                    if layer == 2:
                        # add gathered as[src] (contiguous per src-half)
                        s4sb = e4p.tile([P, tb_max * HEADS], F32, tag="s4sb")
                        for h, th, pos in ((0, t0, 0), (1, t1, t0)):
                            if th == 0:
                                continue
                            ob = off_base[h]
                            nc.vector.tensor_tensor(
                                out=s4sb[:, pos * HEADS:(pos + th) * HEADS]
                                    .rearrange("p (t j) -> p t j", j=HEADS),
                                in0=gs[h][:, ob:ob + th, C:C + 4],
                                in1=s4[:, pos * HEADS:(pos + th) * HEADS]
                                    .rearrange("p (t j) -> p t j", j=HEADS),
                                op=mybir.AluOpType.add)
                        s4v = s4sb[:, 0:n4]
                    t4b = e4p.tile([P, tb_max * HEADS], F32, tag="t4b")
                    nc.vector.tensor_scalar_mul(t4b[:, 0:n4], s4v, NEG_SLOPE)
                    t4a = e4p.tile([P, tb_max * HEADS], F32, tag="t4a")
                    nc.vector.tensor_tensor(
                        t4a[:, 0:n4], s4v, t4b[:, 0:n4],
                        op=mybir.AluOpType.max)
                    nc.scalar.activation(
                        m_blk[:, 0:ntile, C:C + 4],
                        t4a[:, 0:n4].rearrange("p (t j) -> p t j", j=HEADS),
                        mybir.ActivationFunctionType.Exp)
                    if layer == 1:
                        nc.vector.tensor_tensor(
                            out=m_blk[:, 0:ntile, 0:C]
                                .rearrange("p t (h c) -> p t h c", h=HEADS),
                            in0=m_blk[:, 0:ntile, 0:C]
                                .rearrange("p t (h c) -> p t h c", h=HEADS),
                            in1=m_blk[:, 0:ntile, C:C + 4].unsqueeze(-1)
                                .to_broadcast([P, ntile, HEADS, HID]),
                            op=mybir.AluOpType.mult)
                    else:
                        for h, th, pos in ((0, t0, 0), (1, t1, t0)):
                            if th == 0:
                                continue
                            ob = off_base[h]
                            nc.vector.tensor_tensor(
                                out=m_blk[:, pos:pos + th, 0:C]
                                    .rearrange("p t (h c) -> p t h c",
                                               h=HEADS),
                                in0=gs[h][:, ob:ob + th, 0:C]
                                    .rearrange("p t (h c) -> p t h c",
                                               h=HEADS),
                                in1=m_blk[:, pos:pos + th, C:C + 4]
                                    .unsqueeze(-1)
                                    .to_broadcast([P, th, HEADS, HID]),
                                op=mybir.AluOpType.mult)
                    if dump and dump.startswith("dm") and b == 0 \
                            and layer == 1:
                        dlt = int(dump[2:])
                        dt_dbg4 = zp.tile([P, P + C + 4], F32, tag="dbg4")
                        nc.vector.tensor_copy(dt_dbg4[:, 0:P], d_blk[:, dlt])
                        nc.vector.tensor_copy(dt_dbg4[:, P:P + C + 4],
                                              m_blk[:, dlt])
                        nc.sync.dma_start(out=dbg[:, 0:P + C + 4],
                                          in_=dt_dbg4[:])
                    # scatter-accumulate: one matmul chain into PSUM
                    psb = pp.tile([P, C + 4], F32, tag="blk", bufs=2)
                    for lt in range(ntile):
                        nc.tensor.matmul(
                            psb[:], d_blk[:, lt], m_blk[:, lt, 0:C + 4],
                            start=(lt == 0), stop=(lt == ntile - 1))
                    if dump == "psb" and b == 0 and layer == 1:
                        dt_dbg3 = zp.tile([P, C + 4], F32, tag="dbg3")
                        nc.vector.tensor_copy(dt_dbg3[:], psb[:])
                        nc.sync.dma_start(out=dbg[:, 0:C + 4], in_=dt_dbg3[:])
                    post_fn(b, psb)

        def normalize(psum, out_ap):
            rden = e4p.tile([P, HEADS], F32, tag="rd")
            nc.vector.tensor_scalar_max(rden[:], psum[:, C:C + 4], 1e-30)
            nc.vector.reciprocal(rden[:], rden[:])
            nc.vector.tensor_tensor(
                out=out_ap.rearrange("p (h c) -> p h c", h=HEADS),
                in0=psum[:, 0:C].rearrange("p (h c) -> p h c", h=HEADS),
                in1=rden[:].unsqueeze(-1).to_broadcast([P, HEADS, HID]),
                op=mybir.AluOpType.mult)

        def elu_inplace(z, width, tag):
            a = zp.tile([P, width], F32, tag=tag + "a")
            nc.vector.tensor_scalar_min(a[:], z, 0.0)
            nc.scalar.activation(a[:], a[:], mybir.ActivationFunctionType.Exp)
            d = zp.tile([P, width], F32, tag=tag + "d")
            nc.vector.tensor_scalar(
                out=d[:], in0=z, scalar1=0.0, scalar2=1.0,
                op0=mybir.AluOpType.max, op1=mybir.AluOpType.subtract)
            nc.vector.tensor_tensor(z, d[:], a[:], op=mybir.AluOpType.add)

        def pair_transpose(zb_bf16_asf32, rows):
            """[128, rows] f32(bf16-pairs) -> psum [rows, 128] transposed."""
            pt = pp.tile([P, P], F32, tag="tp")
            nc.tensor.transpose(pt[0:rows, :], zb_bf16_asf32, ident[:])
            return pt

        def post1(b, psb):
            z = zp.tile([P, C], F32, tag="z1")
            if psb is None:
                nc.vector.memset(z[:], 0.0)
            else:
                normalize(psb, z[:])
            nc.vector.tensor_tensor(z[:], z[:], b1_t[:], op=mybir.AluOpType.add)
            elu_inplace(z[:], C, "e1")
            if dump == "z1" and b == 0:
                nc.sync.dma_start(out=dbg[:, 0:C], in_=z[:])
            zb = zp.tile([P, C], BF16, tag="zb1")
            nc.vector.tensor_copy(zb[:], z[:])
            ptp = pair_transpose(zb[:].bitcast(F32), P)
            zT4 = zp.tile([P, P], F32, tag="zT4")
            nc.vector.tensor_copy(zT4[:], ptp[:])
            zT4b = zT4[:].bitcast(BF16).rearrange("p (n two) -> p n two", two=2)
            ps2 = pp.tile([P, C + 8], F32, tag="t2b")
            nc.tensor.matmul(ps2[:], zT4b[:, :, 0], w2e_t[:],
                             start=True, stop=False)
            nc.tensor.matmul(ps2[:], zT4b[:, :, 1], w2o_t[:],
                             start=False, stop=True)
            st2 = zp.tile([P, C + 4], BF16, tag="st2")
            nc.vector.tensor_copy(st2[:], ps2[:, 0:C + 4])
            nc.vector.tensor_copy(av2[:, b * HEADS:(b + 1) * HEADS],
                                  ps2[:, C + 4:C + 8])
            nc.sync.dma_start(out=t2_slice[b * P:(b + 1) * P, 0:C + 4],
                              in_=st2[:])

        def post2(b, psb):
            zn = zp.tile([P, C], F32, tag="z2n")
            if psb is None:
                nc.vector.memset(zn[:], 0.0)
            else:
                normalize(psb, zn[:])
            hm = zp.tile([P, HID], F32, tag="hm")
            nc.vector.tensor_reduce(
                out=hm[:],
                in_=zn[:].rearrange("p (h c) -> p c h", h=HEADS),
                axis=mybir.AxisListType.X, op=mybir.AluOpType.add)
            nc.vector.tensor_scalar_mul(hm[:], hm[:], 1.0 / HEADS)
            nc.vector.tensor_tensor(hm[:], hm[:], b2_t[:],
                                    op=mybir.AluOpType.add)
            elu_inplace(hm[:], HID, "e2")
            hb = zp.tile([P, HID], BF16, tag="hb2")
            nc.vector.tensor_copy(hb[:], hm[:])
            ptp = pair_transpose(hb[:].bitcast(F32), HID // 2)
            zT2 = zp.tile([HID // 2, P], F32, tag="zT2")
            nc.vector.tensor_copy(zT2[:], ptp[0:HID // 2, :])
            zT2b = zT2[:].bitcast(BF16).rearrange("p (n two) -> p n two", two=2)
            psy = pp.tile([P, OUT_CH], F32, tag="tp")
            nc.tensor.matmul(psy[:], zT2b[:, :, 0], wce_t[:],
                             start=True, stop=False)
            nc.tensor.matmul(psy[:], zT2b[:, :, 1], wco_t[:],
                             start=False, stop=True)
            yt = zp.tile([P, OUT_CH], F32, tag="yt")
            nc.vector.tensor_tensor(yt[:], psy[:], bc_t[:],
                                    op=mybir.AluOpType.add)
            nc.sync.dma_start(out=outs["y"][b * P:(b + 1) * P, :], in_=yt[:])

        # ---- P-B: layer-1 edge pass (builds t2_slice and av2 in post1)
        edge_pass(1, post1)

        if phases < 3:
            return
        # ---- P-C: AllGather layer-2 table
        nc.gpsimd.collective_compute(
            "AllGather", mybir.AluOpType.bypass,
            replica_groups=[list(range(NC))],
            ins=[t2_slice[:]], outs=[t2_full[:]],
        )

        if phases < 4:
            return
        # ---- P-D: layer-2 edge pass
        edge_pass(2, post2)


# ----------------------------------------------------------------------------
# entry point
# ----------------------------------------------------------------------------

def _prepare(inputs, n_nodes, npc):
    ei = np.asarray(inputs["edge_index"])
    src = np.concatenate([ei[0], np.arange(n_nodes, dtype=ei.dtype)])
    src = src.astype(np.int64)
    dst = np.concatenate([ei[1], np.arange(n_nodes, dtype=ei.dtype)])
    dst = dst.astype(np.int64)
    meta, per_core = _prep_edges(src, dst, npc)
    npad = meta["npad"]

    x = np.asarray(inputs["x"], np.float32)
    xtab = np.zeros((npad, P), np.float32)
    xtab[:n_nodes, 0:IN_CH] = x
    xtab = _bf16(xtab)
    xT = np.zeros((IN_CH, npad), np.float32)
    xT[:, :n_nodes] = x.T
    xT = _bf16(xT)

    W1 = np.asarray(inputs["W1"], np.float32)
    W2 = np.asarray(inputs["W2"], np.float32)
    W1av = _bf16(np.concatenate(
        [W1, W1 @ _fold_as(np.asarray(inputs["as1"], np.float32))], axis=1))
    W1Ad = _bf16(W1 @ _fold_as(np.asarray(inputs["ad1"], np.float32)))
    W2avd = np.concatenate(
        [W2, W2 @ _fold_as(np.asarray(inputs["as2"], np.float32)),
         W2 @ _fold_as(np.asarray(inputs["ad2"], np.float32))], axis=1)
    W2avdE = _bf16(W2avd[0::2])
    W2avdO = _bf16(W2avd[1::2])
    Wc = np.asarray(inputs["Wc"], np.float32)
    b1r = np.tile(np.asarray(inputs["b1"], np.float32)[None, :], (P, 1))
    b2r = np.tile(np.asarray(inputs["b2"], np.float32)[None, :], (P, 1))
    bcr = np.tile(np.asarray(inputs["bc"], np.float32)[None, :], (P, 1))
    iota = np.tile(np.arange(P, dtype=np.float32)[None, :], (P, 1))

    in_maps = []
    for k in range(NC):
        m = {
            "xtab": xtab,
            "xTloc": np.ascontiguousarray(xT[:, k * npc:(k + 1) * npc]),
            "W1av": W1av, "W1Ad": W1Ad,
            "W2avdE": W2avdE, "W2avdO": W2avdO,
            "WcE": _bf16(Wc[0::2]), "WcO": _bf16(Wc[1::2]),
            "b1r": b1r, "b2r": b2r, "bcr": bcr, "iota": iota,
            "srcidx": per_core[k]["srcidx"],
            "dlocc": per_core[k]["dlocc"],
            "DT": per_core[k]["DT"],
            "Dm": per_core[k]["Dm"],
        }
        in_maps.append(m)
    return meta, in_maps


def _declare_and_build(nc, meta, sample_map):
    ins = {}
    for name, arr in sample_map.items():
        ins[name] = nc.dram_tensor(
            name, list(arr.shape), mybir.dt.from_np(arr.dtype),
            kind="ExternalInput").ap()
    y = nc.dram_tensor("y", [meta["npc"], OUT_CH], F32, kind="ExternalOutput")
    outs = {"y": y.ap()}
    if meta.get("dump"):
        dbg = nc.dram_tensor("dbg", [P, 512], F32, kind="ExternalOutput")
        outs["dbg"] = dbg.ap()
    with tile.TileContext(nc) as tc:
        build_gat(tc, outs, ins, meta)
    nc.compile()


TRACE = False
LAST_RESULT = None
PHASES = 4
DUMP = None
CORES = NC


def kernel(**inputs) -> np.ndarray:
    global LAST_RESULT
    from concourse.bass_utils import run_bass_kernel_spmd

    n_nodes = inputs["x"].shape[0]
    npc = -(-n_nodes // (NC * P)) * P
    meta, in_maps = _prepare(inputs, n_nodes, npc)
    meta["phases"] = PHASES
    meta["dump"] = DUMP

    nc = bacc.Bacc("TRN2", target_bir_lowering=False, num_swdge_queues=NQ)
    _declare_and_build(nc, meta, in_maps[0])

    res = run_bass_kernel_spmd(nc, in_maps[:CORES], core_ids=list(range(CORES)),
                               trace=TRACE)
    LAST_RESULT = res
    y = np.concatenate([r["y"] for r in res.results], axis=0)[:n_nodes]
    return y.astype(np.float32)


# revision 31
# speedup vs baseline: 2.2398x; 1.1199x over previous
"""GAT (2-layer, PyG-style) Trainium2 Bass kernel, 8-core SPMD.

Destination-node partitioning: each core owns a contiguous range of dst nodes
and all edges into it (host pre-groups edges by (dst block, src half)).

Layer 1 gathers raw x rows (padded to 256B, bf16) with dma_gather
transpose=True so they arrive channel-partitioned, then expands h|a_src per
edge with a K=16 matmul against W1|W1@As (no node table, no AllGather).
Layer 2 builds a per-node table [h2|as2] (768B bf16 rows) during layer-1
post-processing, AllGathers it once, and edge-gathers rows directly.

Per edge tile (128 edges): a_dst arrives via a small DT@av matmul (DT = host
one-hot dst transpose), e4 = exp(leakyrelu(as+ad)) is fused into the message
multiply, and a device-generated one-hot D (iota is_equal dloc) turns
scatter-add into PSUM matmul accumulation with softmax denominators riding as
4 extra rhs columns. Gathers round-robin over 4 SWDGE queues so descriptor
generation runs on all four Q7 cpu pairs concurrently.
"""

from contextlib import ExitStack

import numpy as np
import ml_dtypes

import concourse.bass as bass
import concourse.bacc as bacc
import concourse.mybir as mybir
import concourse.tile as tile
from concourse.masks import make_identity

P = 128
NC = 8
IN_CH = 16
HEADS = 4
HID = 64
C = HEADS * HID          # 256
OUT_CH = 8
ELEM2 = 384              # L2 table row: h(256) | as(4) | pad -> 384 bf16 = 768 B
NEG_SLOPE = 0.2
SB = 2                   # dst blocks per gather call
NQ = 4                   # SWDGE queues
F32 = mybir.dt.float32
BF16 = mybir.dt.bfloat16
I16 = mybir.dt.int16

BF1 = np.uint16(0x3F80)  # 1.0 in bf16 bits


def _bf16(x):
    return np.asarray(x, ml_dtypes.bfloat16).view(np.uint16)


# ----------------------------------------------------------------------------
# host-side preprocessing
# ----------------------------------------------------------------------------

def _prep_edges(src, dst, npc):
    """Group edges per core by (dst block, src half); build shared tile meta
    plus per-core index/dloc/DT arrays."""
    npad = NC * npc
    half = npad // 2
    nb = npc // P
    assert npc % P == 0 and half <= 32768

    core_of = dst // npc
    per_core = []
    counts = np.zeros((NC, nb, 2), np.int64)
    for k in range(NC):
        sel = core_of == k
        s = src[sel]
        dl = dst[sel] - k * npc
        blk = dl >> 7
        hlf = s // half
        order = np.lexsort((s, hlf, blk))
        s, dl, blk, hlf = s[order], dl[order], blk[order], hlf[order]
        np.add.at(counts[k], (blk, hlf), 1)
        # group start offsets in sorted arrays
        gstart = np.zeros((nb, 2), np.int64)
        gcnt = np.zeros((nb, 2), np.int64)
        idx = 0
        for b in range(nb):
            for h in range(2):
                cnt = int(((blk == b) & (hlf == h)).sum())
                gstart[b, h] = idx
                gcnt[b, h] = cnt
                idx += cnt
        per_core.append((s, dl, gstart, gcnt))

    T = np.ceil(counts.max(axis=0) / P).astype(np.int64)   # [nb, 2]
    TT = int(T.sum())
    tb = T.sum(axis=1)                                     # tiles per block
    tb_max = int(tb.max())

    nsb = -(-nb // SB)
    # calls: (sb, h) -> col start (in 16-wrapped units), nidx
    calls = []
    cs = 0
    call_id = {}
    for sb in range(nsb):
        blocks = list(range(sb * SB, min((sb + 1) * SB, nb)))
        for h in range(2):
            nidx = int(sum(T[b, h] for b in blocks) * P)
            call_id[(sb, h)] = len(calls)
            calls.append({"sb": sb, "h": h, "cs": cs, "nidx": nidx,
                          "blocks": blocks})
            cs += nidx // 16
    CT = cs

    # processing order tiles: for sb, for b in sb, for h, for tile
    tiles = []           # (b, h, call, off_in_call, pt)
    pt = 0
    pt_start = np.zeros(nb + 1, np.int64)
    for sb in range(nsb):
        blocks = calls[call_id[(sb, 0)]]["blocks"]
        for bi, b in enumerate(blocks):
            pt_start[b] = pt
            for h in range(2):
                off = int(sum(T[bb, h] for bb in blocks[:bi]))
                for i in range(int(T[b, h])):
                    tiles.append((b, h, call_id[(sb, h)], off + i, pt))
                    pt += 1
    pt_start[nb] = pt
    assert pt == TT

    meta = {"npc": npc, "npad": npad, "half": half, "nb": nb, "nsb": nsb,
            "T": T, "TT": TT, "tb": tb, "tb_max": tb_max, "calls": calls,
            "tiles": tiles, "pt_start": pt_start, "CT": CT}

    per_core_arrays = []
    for k in range(NC):
        s, dl, gstart, gcnt = per_core[k]
        idx16 = np.zeros((16, CT), np.int16)
        dloc = np.full(TT * P, -1, np.int64)
        gpos = np.zeros((nb, 2), np.int64)   # consumed edges per group
        for (b, h, c, off, ptt) in tiles:
            call = calls[c]
            g0 = int(gstart[b, h]) + int(gpos[b, h])
            n = min(int(gcnt[b, h]) - int(gpos[b, h]), P)
            gpos[b, h] += n
            if n <= 0:
                continue
            sl = np.arange(n)
            j = off * P + sl                      # slot within call
            col = call["cs"] + j // 16
            idx16[j % 16, col] = (s[g0:g0 + n] % half).astype(np.int16)
            dloc[ptt * P + sl] = dl[g0:g0 + n] & 127

        DT = np.zeros((P, TT * P), np.uint16)
        valid = dloc >= 0
        vs = np.where(valid)[0]
        DT[dloc[valid], vs] = BF1
        Dm = np.zeros((P, TT * P), np.uint16)
        Dm[vs % P, (vs // P) * P + dloc[valid]] = BF1

        per_core_arrays.append({
            "srcidx": np.tile(idx16, (8, 1)),
            "dlocc": np.ascontiguousarray(
                dloc.reshape(TT, P).T.astype(np.float32)),
            "DT": DT,
            "Dm": Dm,
        })
    return meta, per_core_arrays


def _fold_as(a_s):
    As = np.zeros((C, HEADS), np.float32)
    for h in range(HEADS):
        As[h * HID:(h + 1) * HID, h] = a_s[h]
    return As


# ----------------------------------------------------------------------------
# device program
# ----------------------------------------------------------------------------

def build_gat(tc, outs, ins, meta):
    nc = tc.nc
    npc, half, nb, nsb = meta["npc"], meta["half"], meta["nb"], meta["nsb"]
    npad = meta["npad"]
    T, calls, tiles = meta["T"], meta["calls"], meta["tiles"]
    tb, tb_max, TT = meta["tb"], meta["tb_max"], meta["TT"]
    pt_start = meta["pt_start"]
    phases = meta.get("phases", 4)

    t2_slice = nc.dram_tensor("t2_slice", [npc, ELEM2], BF16)
    t2_full = nc.dram_tensor("t2_full", [npad, ELEM2], BF16,
                             addr_space="Shared")
    dump = meta.get("dump")
    dbg = outs.get("dbg")

    with ExitStack() as ctx:
        consts = ctx.enter_context(tc.tile_pool(name="consts", bufs=1))
        gp = ctx.enter_context(tc.tile_pool(name="gp", bufs=4))
        dtp = ctx.enter_context(tc.tile_pool(name="dtp", bufs=2))
        dp = ctx.enter_context(tc.tile_pool(name="dp", bufs=2))
        mp = ctx.enter_context(tc.tile_pool(name="mp", bufs=2))
        zp = ctx.enter_context(tc.tile_pool(name="zp", bufs=2))
        e4p = ctx.enter_context(tc.tile_pool(name="e4p", bufs=2))
        pp = ctx.enter_context(tc.tile_pool(name="pp", bufs=1, space="PSUM"))

        # ---- constants
        idx_t = consts.tile([P, meta["CT"]], I16)
        nc.sync.dma_start(out=idx_t[:], in_=ins["srcidx"][:])
        w1_t = consts.tile([IN_CH, C + 4], BF16)
        nc.sync.dma_start(out=w1_t[:], in_=ins["W1av"][:].bitcast(BF16))
        w1ad_t = consts.tile([IN_CH, HEADS], BF16)
        nc.sync.dma_start(out=w1ad_t[:], in_=ins["W1Ad"][:].bitcast(BF16))
        w2e_t = consts.tile([P, C + 8], BF16)
        nc.sync.dma_start(out=w2e_t[:], in_=ins["W2avdE"][:].bitcast(BF16))
        w2o_t = consts.tile([P, C + 8], BF16)
        nc.sync.dma_start(out=w2o_t[:], in_=ins["W2avdO"][:].bitcast(BF16))
        wce_t = consts.tile([HID // 2, OUT_CH], BF16)
        nc.sync.dma_start(out=wce_t[:], in_=ins["WcE"][:].bitcast(BF16))
        wco_t = consts.tile([HID // 2, OUT_CH], BF16)
        nc.sync.dma_start(out=wco_t[:], in_=ins["WcO"][:].bitcast(BF16))
        b1_t = consts.tile([P, C], F32)
        nc.sync.dma_start(out=b1_t[:], in_=ins["b1r"][:])
        b2_t = consts.tile([P, HID], F32)
        nc.sync.dma_start(out=b2_t[:], in_=ins["b2r"][:])
        bc_t = consts.tile([P, OUT_CH], F32)
        nc.sync.dma_start(out=bc_t[:], in_=ins["bcr"][:])
        ident = consts.tile([P, P], F32)
        make_identity(nc, ident[:])
        av1 = consts.tile([P, nb * HEADS], BF16)
        av2 = consts.tile([P, nb * HEADS], BF16)

        nregs = {}
        for c in calls:
            if c["nidx"] and c["nidx"] not in nregs:
                nregs[c["nidx"]] = nc.gpsimd.to_reg(c["nidx"])

        # ---- P-A: av1[n] = x[n] @ (W1@Ad1)  (block-local a_dst table)
        for b in range(nb):
            xTb = zp.tile([IN_CH, P], BF16, tag="xTb")
            nc.sync.dma_start(
                out=xTb[:], in_=ins["xTloc"][:, b * P:(b + 1) * P]
                    .bitcast(BF16))
            ps = pp.tile([P, C], F32, tag="e", bufs=2)
            nc.tensor.matmul(ps[:, 0:HEADS], xTb[:], w1ad_t[:],
                             start=True, stop=True)
            nc.vector.tensor_copy(av1[:, b * HEADS:(b + 1) * HEADS],
                                  ps[:, 0:HEADS])

        if phases < 2:
            return

        qrr = [0]

        def gather(c, layer):
            if layer == 1:
                q = 0          # transpose gathers share the xbar; serialize
            else:
                q = qrr[0] % NQ
                qrr[0] += 1
            nidx = c["nidx"]
            if nidx == 0:
                return None
            cs = c["cs"]
            h = c["h"]
            if layer == 1:
                g = gp.tile([P, 1, nidx], BF16, tag=f"g1{h}", bufs=4)
                nc.gpsimd.dma_gather(
                    out_ap=g[:], in_ap=ins["xtab"][h * half:(h + 1) * half, :]
                        .bitcast(BF16),
                    idxs_ap=idx_t[:, cs:cs + nidx // 16],
                    num_idxs=nidx, num_idxs_reg=nregs[nidx],
                    elem_size=P, transpose=True, single_packet=False,
                    queue_num=q)
            else:
                g = gp.tile([P, nidx // P, ELEM2], BF16, tag=f"g2{h}", bufs=2)
                nc.gpsimd.dma_gather(
                    out_ap=g[:], in_ap=t2_full[h * half:(h + 1) * half, :],
                    idxs_ap=idx_t[:, cs:cs + nidx // 16],
                    num_idxs=nidx, num_idxs_reg=nregs[nidx],
                    elem_size=ELEM2, transpose=False, single_packet=False,
                    queue_num=q)
            return g

        def edge_pass(layer, post_fn):
            ti = 0
            for sb in range(nsb):
                c0 = calls[2 * sb]
                c1 = calls[2 * sb + 1]
                g0 = gather(c0, layer)
                g1 = gather(c1, layer)
                gs = (g0, g1)
                for bi, b in enumerate(c0["blocks"]):
                    ntile = int(tb[b])
                    if ntile == 0:
                        post_fn(b, None)
                        continue
                    p0 = int(pt_start[b])
                    t0 = int(T[b, 0])
                    t1 = int(T[b, 1])
                    off_base = [int(sum(T[bb, hh] for bb in
                                        c0["blocks"][:bi])) for hh in (0, 1)]
                    dt_t = dtp.tile([P, tb_max * P], BF16, tag="dt")
                    nc.sync.dma_start(
                        out=dt_t[:, 0:ntile * P],
                        in_=ins["DT"][:, p0 * P:(p0 + ntile) * P]
                            .bitcast(BF16))
                    d_blk = dp.tile([P, tb_max, P], BF16, tag="d")
                    nc.sync.dma_start(
                        out=d_blk[:, 0:ntile, :],
                        in_=ins["Dm"][:, p0 * P:(p0 + ntile) * P]
                            .bitcast(BF16))
                    m_blk = mp.tile([P, tb_max, C + 4], BF16, tag="m")
                    s4 = pp.tile([P, tb_max * HEADS], F32, tag="s4", bufs=2)
                    avs = (av1 if layer == 1 else av2)[
                        :, b * HEADS:(b + 1) * HEADS]
                    for lt in range(ntile):
                        bb, h, ci, off, ptt = tiles[ti]
                        assert bb == b and ptt == p0 + lt
                        ti += 1
                        g = gs[h]
                        r4 = s4[:, lt * HEADS:(lt + 1) * HEADS]
                        dts = dt_t[:, lt * P:(lt + 1) * P]
                        nc.tensor.matmul(r4, dts, avs, start=(lt == 0),
                                            stop=(lt == ntile - 1))
                        if layer == 1:
                            xg = g[0:IN_CH, 0, off * P:(off + 1) * P]
                            pse = pp.tile([P, C + 4], F32, tag="e", bufs=2)
                            nc.tensor.matmul(pse[:], xg, w1_t[:],
                                             start=True, stop=True)
                            # evict h|as to m via scalar engine (bf16 cast)
                            nc.scalar.activation(
                                m_blk[:, lt, 0:C + 4], pse[:],
                                mybir.ActivationFunctionType.Copy)
                    # ---- batched per-block e4 + fold
                    n4 = ntile * HEADS
                    s4sb = e4p.tile([P, tb_max * HEADS], F32, tag="s4sb")
                    if layer == 1:
                        nc.vector.tensor_tensor(
                            out=s4sb[:, 0:n4]
                                .rearrange("p (t j) -> p t j", j=HEADS),
                            in0=m_blk[:, 0:ntile, C:C + 4],
                            in1=s4[:, 0:n4]
                                .rearrange("p (t j) -> p t j", j=HEADS),
                            op=mybir.AluOpType.add)
                    else:
                        for h, th, pos in ((0, t0, 0), (1, t1, t0)):
                            if th == 0:
                                continue
                            ob = off_base[h]
                            nc.vector.tensor_tensor(
                                out=s4sb[:, pos * HEADS:(pos + th) * HEADS]
                                    .rearrange("p (t j) -> p t j", j=HEADS),
                                in0=gs[h][:, ob:ob + th, C:C + 4],
                                in1=s4[:, pos * HEADS:(pos + th) * HEADS]
                                    .rearrange("p (t j) -> p t j", j=HEADS),
                                op=mybir.AluOpType.add)
                    s4v = s4sb[:, 0:n4]
                    t4b = e4p.tile([P, tb_max * HEADS], F32, tag="t4b")
                    nc.vector.tensor_scalar_mul(t4b[:, 0:n4], s4v, NEG_SLOPE)
                    t4a = e4p.tile([P, tb_max * HEADS], F32, tag="t4a")
                    nc.vector.tensor_tensor(
                        t4a[:, 0:n4], s4v, t4b[:, 0:n4],
                        op=mybir.AluOpType.max)
                    nc.scalar.activation(
                        m_blk[:, 0:ntile, C:C + 4],
                        t4a[:, 0:n4].rearrange("p (t j) -> p t j", j=HEADS),
                        mybir.ActivationFunctionType.Exp)
                    if layer == 1:
                        nc.vector.tensor_tensor(
                            out=m_blk[:, 0:ntile, 0:C]
                                .rearrange("p t (h c) -> p t h c", h=HEADS),
                            in0=m_blk[:, 0:ntile, 0:C]
                                .rearrange("p t (h c) -> p t h c", h=HEADS),
                            in1=m_blk[:, 0:ntile, C:C + 4].unsqueeze(-1)
                                .to_broadcast([P, ntile, HEADS, HID]),
                            op=mybir.AluOpType.mult)
                    else:
                        for h, th, pos in ((0, t0, 0), (1, t1, t0)):
                            if th == 0:
                                continue
                            ob = off_base[h]
                            nc.vector.tensor_tensor(
                                out=m_blk[:, pos:pos + th, 0:C]
                                    .rearrange("p t (h c) -> p t h c",
                                               h=HEADS),
                                in0=gs[h][:, ob:ob + th, 0:C]
                                    .rearrange("p t (h c) -> p t h c",
                                               h=HEADS),
                                in1=m_blk[:, pos:pos + th, C:C + 4]
                                    .unsqueeze(-1)
                                    .to_broadcast([P, th, HEADS, HID]),
                                op=mybir.AluOpType.mult)
                    if dump and dump.startswith("dm") and b == 0 \
                            and layer == 1:
                        dlt = int(dump[2:])
                        dt_dbg4 = zp.tile([P, P + C + 4], F32, tag="dbg4")
                        nc.vector.tensor_copy(dt_dbg4[:, 0:P], d_blk[:, dlt])
                        nc.vector.tensor_copy(dt_dbg4[:, P:P + C + 4],
                                              m_blk[:, dlt])
                        nc.sync.dma_start(out=dbg[:, 0:P + C + 4],
                                          in_=dt_dbg4[:])
                    # scatter-accumulate: one matmul chain into PSUM
                    psb = pp.tile([P, C + 4], F32, tag="blk", bufs=2)
                    for lt in range(ntile):
                        nc.tensor.matmul(
                            psb[:], d_blk[:, lt], m_blk[:, lt, 0:C + 4],
                            start=(lt == 0), stop=(lt == ntile - 1))
                    if dump == "psb" and b == 0 and layer == 1:
                        dt_dbg3 = zp.tile([P, C + 4], F32, tag="dbg3")
                        nc.vector.tensor_copy(dt_dbg3[:], psb[:])
                        nc.sync.dma_start(out=dbg[:, 0:C + 4], in_=dt_dbg3[:])
                    post_fn(b, psb)

        def evict(psum, width, tag):
            """PSUM -> SBUF f32 via the scalar engine."""
            t = zp.tile([P, width], F32, tag=tag)
            nc.scalar.activation(t[:], psum,
                                 mybir.ActivationFunctionType.Copy)
            return t

        def normalize(zsb, out_ap):
            """out = zsb[:, 0:C] / broadcast(max(zsb[:, C:C+4], eps))"""
            rden = e4p.tile([P, HEADS], F32, tag="rd")
            nc.vector.tensor_scalar_max(rden[:], zsb[:, C:C + 4], 1e-30)
            nc.vector.reciprocal(rden[:], rden[:])
            nc.vector.tensor_tensor(
                out=out_ap.rearrange("p (h c) -> p h c", h=HEADS),
                in0=zsb[:, 0:C].rearrange("p (h c) -> p h c", h=HEADS),
                in1=rden[:].unsqueeze(-1).to_broadcast([P, HEADS, HID]),
                op=mybir.AluOpType.mult)
            return rden

        def elu_to_bf16(z, out_bf, width, tag):
            """out_bf = elu(z) in bf16: relu(z) + exp(-relu(-z)) - 1."""
            u = zp.tile([P, width], F32, tag=tag + "u")
            nc.scalar.activation(u[:], z,
                                 mybir.ActivationFunctionType.Relu,
                                 scale=-1.0)
            nc.scalar.activation(u[:], u[:],
                                 mybir.ActivationFunctionType.Exp,
                                 scale=-1.0)
            v = zp.tile([P, width], F32, tag=tag + "v")
            nc.scalar.activation(v[:], z,
                                 mybir.ActivationFunctionType.Relu)
            nc.vector.tensor_tensor(v[:], v[:], u[:],
                                    op=mybir.AluOpType.add)
            nc.vector.tensor_scalar_add(out_bf, v[:], -1.0)

        def pair_transpose(zb_bf16_asf32, rows):
            """[128, rows] f32(bf16-pairs) -> psum [rows, 128] transposed."""
            pt = pp.tile([P, P], F32, tag="tp")
            nc.tensor.transpose(pt[0:rows, :], zb_bf16_asf32, ident[:])
            return pt

        def post1(b, psb):
            zb = zp.tile([P, C], BF16, tag="zb1")
            if psb is None:
                nc.vector.memset(zb[:], 0.0)
            else:
                zsb = evict(psb[:], C + 4, "zs1")
                z = zp.tile([P, C], F32, tag="z1")
                normalize(zsb, z[:])
                nc.vector.tensor_tensor(z[:], z[:], b1_t[:],
                                        op=mybir.AluOpType.add)
                elu_to_bf16(z[:], zb[:], C, "e1")
            if dump == "z1" and b == 0:
                zdbg = zp.tile([P, C], F32, tag="zdbg")
                nc.vector.tensor_copy(zdbg[:], zb[:])
                nc.sync.dma_start(out=dbg[:, 0:C], in_=zdbg[:])
            ptp = pair_transpose(zb[:].bitcast(F32), P)
            zT4 = evict(ptp[:], P, "zT4")
            zT4b = zT4[:].bitcast(BF16).rearrange("p (n two) -> p n two", two=2)
            ps2 = pp.tile([P, C + 8], F32, tag="t2b")
            nc.tensor.matmul(ps2[:], zT4b[:, :, 0], w2e_t[:],
                             start=True, stop=False)
            nc.tensor.matmul(ps2[:], zT4b[:, :, 1], w2o_t[:],
                             start=False, stop=True)
            st2 = zp.tile([P, C + 4], BF16, tag="st2")
            nc.scalar.activation(st2[:], ps2[:, 0:C + 4],
                                 mybir.ActivationFunctionType.Copy)
            nc.scalar.activation(av2[:, b * HEADS:(b + 1) * HEADS],
                                 ps2[:, C + 4:C + 8],
                                 mybir.ActivationFunctionType.Copy)
            nc.sync.dma_start(out=t2_slice[b * P:(b + 1) * P, 0:C + 4],
                              in_=st2[:])

        def post2(b, psb):
            hb = zp.tile([P, HID], BF16, tag="hb2")
            if psb is None:
                nc.vector.memset(hb[:], 0.0)
            else:
                zsb = evict(psb[:], C + 4, "zs2")
                rden = e4p.tile([P, HEADS], F32, tag="rd")
                nc.vector.tensor_scalar_max(rden[:], zsb[:, C:C + 4], 1e-30)
                nc.vector.reciprocal(rden[:], rden[:])
                nc.vector.tensor_scalar_mul(rden[:], rden[:], 1.0 / HEADS)
                zn = zp.tile([P, C], F32, tag="z2n")
                nc.vector.tensor_tensor(
                    out=zn[:].rearrange("p (h c) -> p h c", h=HEADS),
                    in0=zsb[:, 0:C].rearrange("p (h c) -> p h c", h=HEADS),
                    in1=rden[:].unsqueeze(-1).to_broadcast([P, HEADS, HID]),
                    op=mybir.AluOpType.mult)
                hm = zp.tile([P, HID], F32, tag="hm")
                nc.vector.tensor_reduce(
                    out=hm[:],
                    in_=zn[:].rearrange("p (h c) -> p c h", h=HEADS),
                    axis=mybir.AxisListType.X, op=mybir.AluOpType.add)
                nc.vector.tensor_tensor(hm[:], hm[:], b2_t[:],
                                        op=mybir.AluOpType.add)
                elu_to_bf16(hm[:], hb[:], HID, "e2")
            ptp = pair_transpose(hb[:].bitcast(F32), HID // 2)
            zT2 = zp.tile([HID // 2, P], F32, tag="zT2")
            nc.scalar.activation(zT2[:], ptp[0:HID // 2, :],
                                 mybir.ActivationFunctionType.Copy)
            zT2b = zT2[:].bitcast(BF16).rearrange("p (n two) -> p n two", two=2)
            psy = pp.tile([P, OUT_CH], F32, tag="tp")
            nc.tensor.matmul(psy[:], zT2b[:, :, 0], wce_t[:],
                             start=True, stop=False)
            nc.tensor.matmul(psy[:], zT2b[:, :, 1], wco_t[:],
                             start=False, stop=True)
            yt = zp.tile([P, OUT_CH], F32, tag="yt")
            nc.vector.tensor_tensor(yt[:], psy[:], bc_t[:],
                                    op=mybir.AluOpType.add)
            nc.sync.dma_start(out=outs["y"][b * P:(b + 1) * P, :], in_=yt[:])

        # ---- P-B: layer-1 edge pass (builds t2_slice and av2 in post1)
        edge_pass(1, post1)

        if phases < 3:
            return
        # ---- P-C: AllGather layer-2 table
        nc.gpsimd.collective_compute(
            "AllGather", mybir.AluOpType.bypass,
            replica_groups=[list(range(NC))],
            ins=[t2_slice[:]], outs=[t2_full[:]],
        )

        if phases < 4:
            return
        # ---- P-D: layer-2 edge pass
        edge_pass(2, post2)


# ----------------------------------------------------------------------------
# entry point
# ----------------------------------------------------------------------------

def _prepare(inputs, n_nodes, npc):
    ei = np.asarray(inputs["edge_index"])
    src = np.concatenate([ei[0], np.arange(n_nodes, dtype=ei.dtype)])
    src = src.astype(np.int64)
    dst = np.concatenate([ei[1], np.arange(n_nodes, dtype=ei.dtype)])
    dst = dst.astype(np.int64)
    meta, per_core = _prep_edges(src, dst, npc)
    npad = meta["npad"]

    x = np.asarray(inputs["x"], np.float32)
    xtab = np.zeros((npad, P), np.float32)
    xtab[:n_nodes, 0:IN_CH] = x
    xtab = _bf16(xtab)
    xT = np.zeros((IN_CH, npad), np.float32)
    xT[:, :n_nodes] = x.T
    xT = _bf16(xT)

    W1 = np.asarray(inputs["W1"], np.float32)
    W2 = np.asarray(inputs["W2"], np.float32)
    W1av = _bf16(np.concatenate(
        [W1, W1 @ _fold_as(np.asarray(inputs["as1"], np.float32))], axis=1))
    W1Ad = _bf16(W1 @ _fold_as(np.asarray(inputs["ad1"], np.float32)))
    W2avd = np.concatenate(
        [W2, W2 @ _fold_as(np.asarray(inputs["as2"], np.float32)),
         W2 @ _fold_as(np.asarray(inputs["ad2"], np.float32))], axis=1)
    W2avdE = _bf16(W2avd[0::2])
    W2avdO = _bf16(W2avd[1::2])
    Wc = np.asarray(inputs["Wc"], np.float32)
    b1r = np.tile(np.asarray(inputs["b1"], np.float32)[None, :], (P, 1))
    b2r = np.tile(np.asarray(inputs["b2"], np.float32)[None, :], (P, 1))
    bcr = np.tile(np.asarray(inputs["bc"], np.float32)[None, :], (P, 1))
    iota = np.tile(np.arange(P, dtype=np.float32)[None, :], (P, 1))

    in_maps = []
    for k in range(NC):
        m = {
            "xtab": xtab,
            "xTloc": np.ascontiguousarray(xT[:, k * npc:(k + 1) * npc]),
            "W1av": W1av, "W1Ad": W1Ad,
            "W2avdE": W2avdE, "W2avdO": W2avdO,
            "WcE": _bf16(Wc[0::2]), "WcO": _bf16(Wc[1::2]),
            "b1r": b1r, "b2r": b2r, "bcr": bcr, "iota": iota,
            "srcidx": per_core[k]["srcidx"],
            "dlocc": per_core[k]["dlocc"],
            "DT": per_core[k]["DT"],
            "Dm": per_core[k]["Dm"],
        }
        in_maps.append(m)
    return meta, in_maps


def _declare_and_build(nc, meta, sample_map):
    ins = {}
    for name, arr in sample_map.items():
        ins[name] = nc.dram_tensor(
            name, list(arr.shape), mybir.dt.from_np(arr.dtype),
            kind="ExternalInput").ap()
    y = nc.dram_tensor("y", [meta["npc"], OUT_CH], F32, kind="ExternalOutput")
    outs = {"y": y.ap()}
    if meta.get("dump"):
        dbg = nc.dram_tensor("dbg", [P, 512], F32, kind="ExternalOutput")
        outs["dbg"] = dbg.ap()
    with tile.TileContext(nc) as tc:
        build_gat(tc, outs, ins, meta)
    nc.compile()


TRACE = False
LAST_RESULT = None
PHASES = 4
DUMP = None
CORES = NC


def kernel(**inputs) -> np.ndarray:
    global LAST_RESULT
    from concourse.bass_utils import run_bass_kernel_spmd

    n_nodes = inputs["x"].shape[0]
    npc = -(-n_nodes // (NC * P)) * P
    meta, in_maps = _prepare(inputs, n_nodes, npc)
    meta["phases"] = PHASES
    meta["dump"] = DUMP

    nc = bacc.Bacc("TRN2", target_bir_lowering=False, num_swdge_queues=NQ)
    _declare_and_build(nc, meta, in_maps[0])

    res = run_bass_kernel_spmd(nc, in_maps[:CORES], core_ids=list(range(CORES)),
                               trace=TRACE)
    LAST_RESULT = res
    y = np.concatenate([r["y"] for r in res.results], axis=0)[:n_nodes]
    return y.astype(np.float32)
